# revision 31
# baseline (speedup 1.0000x reference)
"""Trainium2 Bass kernel for nn_ChunkProcessor (segment-mean -> 2-layer
transformer encoder over chunks -> gather-expand -> final LN).

Sharding: data-parallel over batch B=8 across the 8 NeuronCores; each core
processes one batch item end to end (no cross-core communication).

v2 vs baseline:
  - tokens + all weights host-cast to bf16 (halves phase-1 HBM traffic).
  - segment counts' reciprocals host-computed (kills 85 tiny PE matmuls).
  - one ACT table set for the whole kernel (natural_log_exp_and_others):
    LN rsqrt = exp(-0.5*ln(var+eps)), softmax denom recip = exp(-ln(x)),
    so no Exp<->Sqrt table thrash between attention and LayerNorm.
  - attention: fused [128,1024] exps, denominator rows batched per 4-head
    group with one ACT ln/exp reciprocal, row-broadcast via GpSimd
    partition_broadcast, numerators multiplied straight out of PSUM.
  - expand one-hots prebuilt during phase 2 (DVE slack), weights single-
    buffered in SBUF with layer-1 refill DMA'd during layer-0 compute.
"""

import numpy as np
import ml_dtypes

B, S, D = 8, 8192, 512
C, H, L, DFF = 512, 8, 2, 2048
HD = D // H          # 64
NT = S // 128        # 64 token tiles
CT = C // 128        # 4 chunk tiles
DT = D // 128        # 4 feature tiles
FT = DFF // 128      # 16
EPS = 1e-5

_CACHE = {}
_DEBUG = False


def _gm_slots(ranges):
    """(group, chunk-tile) pairs the expand phase touches, in emit order."""
    slots = []
    for g in range(NT // 4):
        g_lo = min(ranges[g * 4 + i][0] for i in range(4))
        g_hi = max(ranges[g * 4 + i][1] for i in range(4))
        for m in range(g_lo, g_hi + 1):
            slots.append((g, m))
    return slots


def _build(flags, ranges):
    import concourse.bass as bass
    import concourse.tile as tile
    from concourse import bacc, mybir
    from concourse.masks import make_identity

    (has_qkv_b, has_out_b, has_ff1_b, has_ff2_b,
     has_ln1, has_ln2, has_fln) = flags

    # first/last contributing token tile per chunk tile (PSUM start/stop)
    first_t = [min(t for t in range(NT) if ranges[t][0] <= m <= ranges[t][1])
               for m in range(CT)]
    last_t = [max(t for t in range(NT) if ranges[t][0] <= m <= ranges[t][1])
              for m in range(CT)]

    gm = _gm_slots(ranges)
    slot = {p: i for i, p in enumerate(gm)}
    n_gm = len(gm)
    prebuild_oh = n_gm <= 48

    f32 = mybir.dt.float32
    bf16 = mybir.dt.bfloat16
    f16 = mybir.dt.float16
    AL = mybir.AluOpType
    AF = mybir.ActivationFunctionType

    nc = bacc.Bacc("TRN2", target_bir_lowering=False)

    tokens = nc.declare_dram_parameter("tokens_bf", [S, D], bf16, isOutput=False)
    seg_col = nc.declare_dram_parameter("seg_col", [128, NT], f32, isOutput=False)
    seg_row = nc.declare_dram_parameter("seg_row", [1, S], f16, isOutput=False)
    iota_row = nc.declare_dram_parameter("iota_row", [128, C], f16, isOutput=False)
    iota_col = nc.declare_dram_parameter("iota_col", [128, CT], f32, isOutput=False)
    rcp_cnt = nc.declare_dram_parameter("rcp_cnt", [128, CT], f32, isOutput=False)
    sel2 = nc.declare_dram_parameter("sel2", [2, 128, 128], f32, isOutput=False)
    wqkvT = nc.declare_dram_parameter("wqkvT", [L, D, 3 * D], bf16, isOutput=False)
    woT = nc.declare_dram_parameter("woT", [L, D, D], bf16, isOutput=False)
    w1T = nc.declare_dram_parameter("w1T", [L, D, DFF], bf16, isOutput=False)
    w2T = nc.declare_dram_parameter("w2T", [L, DFF, D], bf16, isOutput=False)
    if has_qkv_b:
        bqkv_c = nc.declare_dram_parameter("bqkv_c", [L, 128, 12], f32, isOutput=False)
        vb_row = nc.declare_dram_parameter("vb_row", [L, 1, D], f32, isOutput=False)
    if has_ff1_b:
        b1_c = nc.declare_dram_parameter("b1_c", [L, 128, FT], f32, isOutput=False)
    if has_out_b:
        outb_row = nc.declare_dram_parameter("outb_row", [L, 1, D], f32, isOutput=False)
    if has_ff2_b:
        ff2b_row = nc.declare_dram_parameter("ff2b_row", [L, 1, D], f32, isOutput=False)
    if has_ln1:
        ln1w_row = nc.declare_dram_parameter("ln1w_row", [L, 1, D], f32, isOutput=False)
        ln1b_row = nc.declare_dram_parameter("ln1b_row", [L, 1, D], f32, isOutput=False)
    if has_ln2:
        ln2w_row = nc.declare_dram_parameter("ln2w_row", [L, 1, D], f32, isOutput=False)
        ln2b_row = nc.declare_dram_parameter("ln2b_row", [L, 1, D], f32, isOutput=False)
    if has_fln:
        flnw_row = nc.declare_dram_parameter("flnw_row", [1, D], f32, isOutput=False)
        flnb_row = nc.declare_dram_parameter("flnb_row", [1, D], f32, isOutput=False)
    out_d = nc.declare_dram_parameter("out", [S, D], f32, isOutput=True)
    if _DEBUG:
        dbg_x0 = nc.declare_dram_parameter("dbg_x0", [128, CT, D], f32,
                                           isOutput=True)
        dbg_xm2 = nc.declare_dram_parameter("dbg_xm2", [128, CT, D], f32,
                                            isOutput=True)
        dbg_oT = nc.declare_dram_parameter("dbg_oT", [128, DT, C], bf16,
                                           isOutput=True)
        dbg_x1 = nc.declare_dram_parameter("dbg_x1", [128, CT, D], f32,
                                           isOutput=True)
        dbg_x2T = nc.declare_dram_parameter("dbg_x2T", [128, DT, C], bf16,
                                            isOutput=True)
        dbg_hT = nc.declare_dram_parameter("dbg_hT", [128, FT, C], bf16,
                                           isOutput=True)

    def bcast_load(pool, dram_row, tag):
        """DMA a [1, D] DRAM row into a [128, D] SBUF tile (partition bcast)."""
        t = pool.tile([128, D], f32, tag=tag, name=f"row_{tag}")
        src = bass.AP(tensor=dram_row.tensor, offset=dram_row.offset,
                      ap=[[0, 128]] + [list(p) for p in dram_row.ap[1:]])
        nc.gpsimd.dma_start(out=t, in_=src)
        return t

    with tile.TileContext(nc) as tc:
        with (
            tc.tile_pool(name="consts", bufs=1) as consts,
            tc.tile_pool(name="acts", bufs=1) as acts,
            tc.tile_pool(name="xm", bufs=2) as xmp,
            tc.tile_pool(name="xt", bufs=2) as xtp,
            tc.tile_pool(name="lnp", bufs=4) as lnp,
            tc.tile_pool(name="rows", bufs=1) as rows,
            tc.tile_pool(name="wts", bufs=1) as wts,
        ):
            # ---------------- constants ----------------
            seg_col_sb = consts.tile([128, NT], f32)
            nc.sync.dma_start(out=seg_col_sb, in_=seg_col[:, :])
            iota_row_sb = consts.tile([128, C], f16)
            nc.sync.dma_start(out=iota_row_sb, in_=iota_row[:, :])
            iota_col_sb = consts.tile([128, CT], f32)
            nc.sync.dma_start(out=iota_col_sb, in_=iota_col[:, :])
            rcp_sb = consts.tile([128, CT], f32)
            nc.sync.dma_start(out=rcp_sb, in_=rcp_cnt[:, :])
            sel_sb = consts.tile([128, 2, 128], f32)
            nc.sync.dma_start(
                out=sel_sb, in_=sel2.rearrange("s p e -> p s e"))
            ident32 = consts.tile([128, 128], f32)
            make_identity(nc, ident32)
            eps_t = consts.tile([128, 1], f32)
            nc.vector.memset(eps_t, EPS)

            # persistent activations
            y_bf = acts.tile([128, CT, D], bf16, tag="y_bf")
            segbc_all = acts.tile([128, NT // 4, 512], f16, tag="segbc")
            if prebuild_oh:
                oh_all = acts.tile([128, n_gm, 512], bf16, tag="oh_all")

            # ---- weights: single-buffered, layer 0 now (scalar HWDGE) ----
            wqkv_sb = wts.tile([128, DT, 3 * D], bf16, tag="wqkv")
            wo_sb = wts.tile([128, DT, D], bf16, tag="wo")
            w1_sb = wts.tile([128, DT, DFF], bf16, tag="w1")
            w2_sb = wts.tile([128, FT, D], bf16, tag="w2")

            def load_weights(l, eng):
                eng.dma_start(
                    out=wqkv_sb, in_=wqkvT[l].rearrange("(dt p) e -> p dt e", p=128))
                eng.dma_start(
                    out=wo_sb, in_=woT[l].rearrange("(dt p) e -> p dt e", p=128))
                eng.dma_start(
                    out=w1_sb, in_=w1T[l].rearrange("(dt p) e -> p dt e", p=128))
                eng.dma_start(
                    out=w2_sb, in_=w2T[l].rearrange("(ft p) e -> p ft e", p=128))

            load_weights(0, nc.scalar)

            # seg broadcast rows for the expand one-hots (SWDGE, idle queue)
            for g in range(NT // 4):
                src = bass.AP(tensor=seg_row[:, :].tensor, offset=g * 512,
                              ap=[[0, 128], [1, 512]])
                nc.gpsimd.dma_start(out=segbc_all[:, g, :], in_=src)

            bqkv_sb, b1_sb = [], []
            vb_sb, outb_sb, ff2b_sb = [], [], []
            ln1w_sb, ln1b_sb, ln2w_sb, ln2b_sb = [], [], [], []
            for l in range(L):
                if has_qkv_b:
                    bq = consts.tile([128, 12], f32, tag=f"bqkv{l}", name=f"bqkv{l}")
                    nc.sync.dma_start(out=bq, in_=bqkv_c[l])
                    bqkv_sb.append(bq)
                    vb_sb.append(bcast_load(rows, vb_row[l], f"vb{l}"))
                if has_ff1_b:
                    b1 = consts.tile([128, FT], f32, tag=f"b1{l}", name=f"b1{l}")
                    nc.sync.dma_start(out=b1, in_=b1_c[l])
                    b1_sb.append(b1)
                if has_out_b:
                    outb_sb.append(bcast_load(rows, outb_row[l], f"outb{l}"))
                if has_ff2_b:
                    ff2b_sb.append(bcast_load(rows, ff2b_row[l], f"ff2b{l}"))
                if has_ln1:
                    ln1w_sb.append(bcast_load(rows, ln1w_row[l], f"ln1w{l}"))
                    ln1b_sb.append(bcast_load(rows, ln1b_row[l], f"ln1b{l}"))
                if has_ln2:
                    ln2w_sb.append(bcast_load(rows, ln2w_row[l], f"ln2w{l}"))
                    ln2b_sb.append(bcast_load(rows, ln2b_row[l], f"ln2b{l}"))
            flnw_sb = bcast_load(rows, flnw_row, "flnw") if has_fln else None
            flnb_sb = bcast_load(rows, flnb_row, "flnb") if has_fln else None

            # ---- batched-LN site: 4 chunk tiles, rsqrt via exp(-.5*ln) ----
            def ln_site(srcs, resids, wrow, brow, outs):
                """outs[m] = LN(srcs[m] (+ resids[m])) [* w + b], token-major."""
                ts, mv4 = [], lnp.tile([128, 2, CT], f32, tag="mv4", name="mv4")
                for m in range(CT):
                    if resids is None:
                        t_ = srcs[m]
                    else:
                        t_ = lnp.tile([128, D], f32, tag="ln_t", name=f"ln_t{m}")
                        nc.vector.tensor_tensor(out=t_, in0=srcs[m],
                                                in1=resids[m], op=AL.add)
                    ts.append(t_)
                    st = lnp.tile([128, 6], f32, tag="ln_st", name="ln_st")
                    nc.vector.bn_stats(out=st, in_=t_)
                    nc.vector.bn_aggr(out=mv4[:, :, m], in_=st)
                # rs = exp(-0.5 * ln(var + eps)) on the contiguous var slice
                ln4 = lnp.tile([128, CT], f32, tag="ln4", name="ln4")
                nc.scalar.activation(out=ln4, in_=mv4[:, 1, :], func=AF.Ln,
                                     bias=eps_t[:, 0:1], scale=1.0)
                rs4 = lnp.tile([128, CT], f32, tag="rs4", name="rs4")
                nc.scalar.activation(out=rs4, in_=ln4, func=AF.Exp,
                                     bias=0.0, scale=-0.5)
                for m in range(CT):
                    if wrow is None:
                        nc.vector.tensor_scalar(
                            out=outs[m], in0=ts[m],
                            scalar1=mv4[:, 0, m:m + 1],
                            scalar2=rs4[:, m:m + 1],
                            op0=AL.subtract, op1=AL.mult)
                    else:
                        xn = lnp.tile([128, D], f32, tag="ln_xn", name="ln_xn")
                        nc.vector.tensor_scalar(
                            out=xn, in0=ts[m],
                            scalar1=mv4[:, 0, m:m + 1],
                            scalar2=rs4[:, m:m + 1],
                            op0=AL.subtract, op1=AL.mult)
                        nc.vector.tensor_tensor(out=xn, in0=xn, in1=wrow, op=AL.mult)
                        nc.vector.tensor_tensor(out=outs[m], in0=xn, in1=brow,
                                                op=AL.add)

            # ------------ phase 1: segment sums (counts from host) ------------
            x0 = xmp.tile([128, CT, D], f32, tag="xm", name="x0")
            with (
                tc.tile_pool(name="pseg", bufs=1, space="PSUM") as pseg,
                tc.tile_pool(name="segs", bufs=5) as segs,
            ):
                ps_sums = [pseg.tile([128, D], f32, tag=f"sums{m}", name=f"sums{m}")
                           for m in range(CT)]
                for t in range(NT):
                    lo, hi = ranges[t]
                    tok = segs.tile([128, D], bf16, tag="tok", name="tok")
                    nc.sync.dma_start(out=tok, in_=tokens[t * 128:(t + 1) * 128, :])
                    oh = segs.tile([128, C], bf16, tag="oh", name="oh")
                    sl = slice(lo * 128, (hi + 1) * 128)
                    nc.vector.tensor_scalar(
                        out=oh[:, sl], in0=iota_row_sb[:, sl],
                        scalar1=seg_col_sb[:, t:t + 1],
                        scalar2=None, op0=AL.is_equal)
                    for m in range(lo, hi + 1):
                        nc.tensor.matmul(
                            ps_sums[m], lhsT=oh[:, m * 128:(m + 1) * 128], rhs=tok,
                            start=(t == first_t[m]), stop=(t == last_t[m]))
                # x = sums * (1/counts)  (host-computed reciprocals)
                for m in range(CT):
                    nc.vector.tensor_scalar(
                        out=x0[:, m, :], in0=ps_sums[m],
                        scalar1=rcp_sb[:, m:m + 1],
                        scalar2=None, op0=AL.mult)

            # one-hot prebuild batches, interleaved at DVE-slack points
            oh_batches = [[] for _ in range(4)]
            if prebuild_oh:
                for i, p in enumerate(gm):
                    oh_batches[i * 4 // n_gm].append(p)

            def emit_oh(batch):
                for (g, m) in batch:
                    nc.vector.tensor_scalar(
                        out=oh_all[:, slot[(g, m)], :], in0=segbc_all[:, g, :],
                        scalar1=iota_col_sb[:, m:m + 1],
                        scalar2=None, op0=AL.is_equal)

            # ---------------- phase 2: transformer ----------------
            x_in = x0
            if _DEBUG:
                nc.sync.dma_start(out=dbg_x0[:, :, :], in_=x0[:, :, :])
            for l in range(L):
                # --- stage A: transposes + qkv projections ---
                with tc.tile_pool(name=f"psqA{l}", bufs=2, space="PSUM") as psq:
                    def transpose_to(src_f32, dst_bf16):
                        for i in range(CT):
                            for j in range(DT):
                                pst = psq.tile([128, 128], f32, tag="ps_t",
                                               name="ps_t")
                                nc.tensor.transpose(
                                    pst, src_f32[:, i, j * 128:(j + 1) * 128],
                                    ident32)
                                nc.vector.tensor_copy(
                                    dst_bf16[:, j, i * 128:(i + 1) * 128], pst)

                    xT = xtp.tile([128, DT, C], bf16, tag="xT", name="xT")
                    transpose_to(x_in, xT)

                    # v token-major [c, e] with per-head ones column (DVE copies)
                    v_ext = acts.tile([128, CT, H, HD + 1], bf16, tag="v_ext",
                                      name="v_ext")
                    nc.vector.memset(v_ext[:, :, :, HD:HD + 1], 1.0)
                    for ct in range(CT):
                        ps = psq.tile([128, C], f32, tag="ps_a", name="ps_a")
                        for dt_ in range(DT):
                            nc.tensor.matmul(
                                ps, lhsT=xT[:, dt_, ct * 128:(ct + 1) * 128],
                                rhs=wqkv_sb[:, dt_, 2 * D:3 * D],
                                start=(dt_ == 0), stop=(dt_ == DT - 1))
                        if has_qkv_b:
                            tv = lnp.tile([128, D], f32, tag="ln_t", name="tv")
                            nc.vector.tensor_tensor(out=tv, in0=ps, in1=vb_sb[l],
                                                    op=AL.add)
                            nc.vector.tensor_copy(v_ext[:, ct, :, 0:HD], tv)
                        else:
                            nc.vector.tensor_copy(v_ext[:, ct, :, 0:HD], ps)

                    # q, k feature-major [e, c] (ACT copies)
                    qT = acts.tile([128, DT, C], bf16, tag="qT", name="qT")
                    kT = acts.tile([128, DT, C], bf16, tag="kT", name="kT")
                    for et in range(8):
                        ps = psq.tile([128, C], f32, tag="ps_a", name="ps_a")
                        for dt_ in range(DT):
                            nc.tensor.matmul(
                                ps, lhsT=wqkv_sb[:, dt_, et * 128:(et + 1) * 128],
                                rhs=xT[:, dt_, :],
                                start=(dt_ == 0), stop=(dt_ == DT - 1))
                        dst = qT[:, et, :] if et < 4 else kT[:, et - 4, :]
                        if has_qkv_b:
                            nc.scalar.activation(
                                out=dst, in_=ps, func=AF.Identity,
                                bias=bqkv_sb[l][:, et:et + 1], scale=1.0)
                        else:
                            nc.scalar.copy(out=dst, in_=ps)

                # --- stage B: attention ---
                oT = acts.tile([128, DT, C], bf16, tag="oT", name="oT")
                with (
                    tc.tile_pool(name=f"pssc{l}", bufs=2, space="PSUM") as scp,
                    tc.tile_pool(name=f"psso{l}", bufs=4, space="PSUM") as psop,
                ):
                    # process heads per PAIR (th = pair): the denominator
                    # recip + broadcast pipeline hides under the next pair's
                    # scores/av, keeping PE dense
                    for pair in range(4):
                        psos = []
                        # den rows restriped to partitions 0 / 32 (32-aligned
                        # DVE offsets); memset keeps junk rows finite
                        den2 = acts.tile([64, C], f32, tag="den2",
                                         name="den2", bufs=2)
                        rcp2 = acts.tile([64, C], f32, tag="rcp2",
                                         name="rcp2", bufs=2)
                        nc.vector.memset(den2, 1.0)
                        for hh in range(2):
                            h = 2 * pair + hh
                            off = hh * 64
                            expT = acts.tile([128, CT * C], bf16, tag="expT",
                                             name="expT", bufs=2)
                            for half in range(2):
                                ps = scp.tile([128, 1024], f32, tag="ps_sc",
                                              name="ps_sc")
                                for k2 in range(2):
                                    kt = half * 2 + k2
                                    nc.tensor.matmul(
                                        ps[:, k2 * 512:(k2 + 1) * 512],
                                        lhsT=kT[off:off + 64, pair,
                                                kt * 128:(kt + 1) * 128],
                                        rhs=qT[off:off + 64, pair, :],
                                        start=True, stop=True)
                                nc.scalar.activation(
                                    out=expT[:, half * 1024:(half + 1) * 1024],
                                    in_=ps, func=AF.Exp, scale=1.0 / 8.0)
                            pso = psop.tile([128, C], f32, tag="ps_o", name="ps_o")
                            for kt in range(CT):
                                nc.tensor.matmul(
                                    pso[0:HD + 1, :], lhsT=v_ext[:, kt, h, :],
                                    rhs=expT[:, kt * 512:(kt + 1) * 512],
                                    start=(kt == 0), stop=(kt == CT - 1))
                            nc.vector.tensor_copy(den2[32 * hh:32 * hh + 1, :],
                                                  pso[HD:HD + 1, :])
                            psos.append(pso)
                        # reciprocal of the 2 den rows: exp(-ln(x))
                        nc.scalar.activation(out=rcp2, in_=den2,
                                             func=AF.Ln, scale=1.0)
                        nc.scalar.activation(out=rcp2, in_=rcp2,
                                             func=AF.Exp, scale=-1.0)
                        # row-broadcast via selector matmul: rb rows
                        # 0:64 <- rcp row 0, 64:128 <- rcp row 32
                        rb_ps = scp.tile([128, 1024], f32, tag="ps_sc",
                                         name="rb_ps")
                        nc.tensor.matmul(rb_ps[:, 0:512],
                                         lhsT=sel_sb[0:64, 0, :],
                                         rhs=rcp2, start=True, stop=True)
                        rb = acts.tile([128, C], f32, tag="rb2",
                                       name="rb2", bufs=2)
                        nc.vector.tensor_copy(rb, rb_ps[:, 0:512])
                        for hh in range(2):
                            off = hh * 64
                            nc.vector.tensor_tensor(
                                out=oT[off:off + 64, pair, :],
                                in0=psos[hh][0:HD, :],
                                in1=rb[off:off + 64, :], op=AL.mult)

                emit_oh(oh_batches[0] if l == 0 else oh_batches[2])

                # --- stage C: out-projection + LN1 + FFN + LN2 ---
                with tc.tile_pool(name=f"psqC{l}", bufs=2, space="PSUM") as psq:
                    xm2 = xmp.tile([128, CT, D], f32, tag="xm", name="xm2")
                    o_ps = []
                    for ct in range(CT):
                        ps = psq.tile([128, C], f32, tag="ps_a", name="ps_a",
                                      bufs=4)
                        for et in range(DT):
                            nc.tensor.matmul(
                                ps, lhsT=oT[:, et, ct * 128:(ct + 1) * 128],
                                rhs=wo_sb[:, et, :],
                                start=(et == 0), stop=(et == DT - 1))
                        if has_out_b:
                            nc.vector.tensor_tensor(out=ps, in0=ps,
                                                    in1=outb_sb[l], op=AL.add)
                        o_ps.append(ps)
                    ln_site(o_ps, [x_in[:, ct, :] for ct in range(CT)],
                            ln1w_sb[l] if has_ln1 else None,
                            ln1b_sb[l] if has_ln1 else None,
                            [xm2[:, ct, :] for ct in range(CT)])

                    def transpose_to2(src_f32, dst_bf16):
                        for i in range(CT):
                            for j in range(DT):
                                pst = psq.tile([128, 128], f32, tag="ps_t",
                                               name="ps_t")
                                nc.tensor.transpose(
                                    pst, src_f32[:, i, j * 128:(j + 1) * 128],
                                    ident32)
                                nc.vector.tensor_copy(
                                    dst_bf16[:, j, i * 128:(i + 1) * 128], pst)

                    x2T = xtp.tile([128, DT, C], bf16, tag="xT", name="x2T")
                    transpose_to2(xm2, x2T)

                    hT = acts.tile([128, FT, C], bf16, tag="hT", name="hT")
                    for ft in range(FT):
                        ps = psq.tile([128, C], f32, tag="ps_a", name="ps_a",
                                      bufs=4)
                        for dt_ in range(DT):
                            nc.tensor.matmul(
                                ps, lhsT=w1_sb[:, dt_, ft * 128:(ft + 1) * 128],
                                rhs=x2T[:, dt_, :],
                                start=(dt_ == 0), stop=(dt_ == DT - 1))
                        if ft % 2 == 0:
                            nc.scalar.activation(
                                out=hT[:, ft, :], in_=ps, func=AF.Relu,
                                bias=(b1_sb[l][:, ft:ft + 1] if has_ff1_b else 0.0),
                                scale=1.0)
                        else:
                            if has_ff1_b:
                                nc.scalar.activation(
                                    out=hT[:, ft, :], in_=ps, func=AF.Relu,
                                    bias=b1_sb[l][:, ft:ft + 1], scale=1.0)
                            else:
                                nc.vector.tensor_scalar(
                                    out=hT[:, ft, :], in0=ps, scalar1=0.0,
                                    scalar2=None, op0=AL.max)
                    x_next = xmp.tile([128, CT, D], f32, tag="xm", name="x_next")
                    f_ps = []
                    for ct in range(CT):
                        ps = psq.tile([128, C], f32, tag="ps_a", name="ps_a",
                                      bufs=4)
                        for ft in range(FT):
                            nc.tensor.matmul(
                                ps, lhsT=hT[:, ft, ct * 128:(ct + 1) * 128],
                                rhs=w2_sb[:, ft, :],
                                start=(ft == 0), stop=(ft == FT - 1))
                        if has_ff2_b:
                            nc.vector.tensor_tensor(out=ps, in0=ps,
                                                    in1=ff2b_sb[l], op=AL.add)
                        f_ps.append(ps)
                    ln_site(f_ps, [xm2[:, ct, :] for ct in range(CT)],
                            ln2w_sb[l] if has_ln2 else None,
                            ln2b_sb[l] if has_ln2 else None,
                            [x_next[:, ct, :] for ct in range(CT)])
                    x_in = x_next
                if _DEBUG and l == 0:
                    nc.sync.dma_start(out=dbg_oT[:, :, :], in_=oT[:, :, :])
                    nc.sync.dma_start(out=dbg_xm2[:, :, :], in_=xm2[:, :, :])
                    nc.sync.dma_start(out=dbg_x1[:, :, :], in_=x_next[:, :, :])
                    nc.sync.dma_start(out=dbg_x2T[:, :, :], in_=x2T[:, :, :])
                    nc.sync.dma_start(out=dbg_hT[:, :, :], in_=hT[:, :, :])

                if l == 0:
                    # refill for layer 1 on the SWDGE queue — emitted after
                    # ALL layer-0 weight reads (out-proj + ffn just above)
                    load_weights(1, nc.gpsimd)
                emit_oh(oh_batches[1] if l == 0 else oh_batches[3])

            # ---------------- phase 3: final LN -> y_bf ----------------
            ln_site([x_in[:, ct, :] for ct in range(CT)], None,
                    flnw_sb, flnb_sb,
                    [y_bf[:, ct, :] for ct in range(CT)])

            # ---------------- expand + write out ----------------
            with (
                tc.tile_pool(name="ohp", bufs=4) as ohp,
                tc.tile_pool(name="outp", bufs=4) as outp,
                tc.tile_pool(name="psE", bufs=4, space="PSUM") as psE,
            ):
                def expand_tile(t, oh_lookup):
                    g, t2 = t // 4, t % 4
                    lo, hi = ranges[t]
                    pse = psE.tile([128, D], f32, tag="ps_e", name="ps_e")
                    for m in range(lo, hi + 1):
                        nc.tensor.matmul(
                            pse, lhsT=oh_lookup(g, m, t2), rhs=y_bf[:, m, :],
                            start=(m == lo), stop=(m == hi))
                    ot = outp.tile([128, D], f32, tag="ot", name="ot")
                    if t % 2 == 0:
                        nc.scalar.copy(out=ot, in_=pse)
                    else:
                        nc.vector.tensor_copy(ot, pse)
                    nc.sync.dma_start(out=out_d[t * 128:(t + 1) * 128, :],
                                      in_=ot)

                if prebuild_oh:
                    # order token tiles by the last chunk tile they need, so
                    # expand for hi==m streams as soon as y_bf[:, m, :] lands
                    for t in sorted(range(NT), key=lambda t: (ranges[t][1], t)):
                        expand_tile(t, lambda g, m, t2: oh_all[
                            :, slot[(g, m)], t2 * 128:(t2 + 1) * 128])
                else:
                    for g in range(NT // 4):
                        g_lo = min(ranges[g * 4 + i][0] for i in range(4))
                        g_hi = max(ranges[g * 4 + i][1] for i in range(4))
                        ohT = ohp.tile([128, CT, 512], bf16, tag="ohT",
                                       name="ohT")
                        for m in range(g_lo, g_hi + 1):
                            nc.vector.tensor_scalar(
                                out=ohT[:, m, :], in0=segbc_all[:, g, :],
                                scalar1=iota_col_sb[:, m:m + 1], scalar2=None,
                                op0=AL.is_equal)
                        for t2 in range(4):
                            expand_tile(g * 4 + t2, lambda g_, m, t2_: ohT[
                                :, m, t2_ * 128:(t2_ + 1) * 128])

    return nc


def _host_prep(inputs):
    """Shard + preprocess full inputs into 8 per-core input maps."""
    bf = ml_dtypes.bfloat16
    tokens = np.asarray(inputs["tokens"], dtype=np.float32)
    seg = np.asarray(inputs["segment_ids"], dtype=np.int32)
    qkv_w = np.asarray(inputs["qkv_w"], dtype=np.float32)
    qkv_b = np.asarray(inputs["qkv_b"], dtype=np.float32)
    out_w = np.asarray(inputs["out_w"], dtype=np.float32)
    out_b = np.asarray(inputs["out_b"], dtype=np.float32)
    ln1_w = np.asarray(inputs["ln1_w"], dtype=np.float32)
    ln1_b = np.asarray(inputs["ln1_b"], dtype=np.float32)
    ln2_w = np.asarray(inputs["ln2_w"], dtype=np.float32)
    ln2_b = np.asarray(inputs["ln2_b"], dtype=np.float32)
    ff1_w = np.asarray(inputs["ff1_w"], dtype=np.float32)
    ff1_b = np.asarray(inputs["ff1_b"], dtype=np.float32)
    ff2_w = np.asarray(inputs["ff2_w"], dtype=np.float32)
    ff2_b = np.asarray(inputs["ff2_b"], dtype=np.float32)
    fln_w = np.asarray(inputs["fln_w"], dtype=np.float32)
    fln_b = np.asarray(inputs["fln_b"], dtype=np.float32)

    flags = (
        bool(np.any(qkv_b)),
        bool(np.any(out_b)),
        bool(np.any(ff1_b)),
        bool(np.any(ff2_b)),
        bool(np.any(ln1_w != 1.0) or np.any(ln1_b)),
        bool(np.any(ln2_w != 1.0) or np.any(ln2_b)),
        bool(np.any(fln_w != 1.0) or np.any(fln_b)),
    )

    # span-bound ranges: per token tile, union over batch of the contiguous
    # chunk-tile range its (sorted) segment ids cover.
    srt = np.all(np.diff(seg, axis=1) >= 0)
    if srt:
        lo = np.min(seg[:, ::128] // 128, axis=0)
        hi = np.max(seg[:, 127::128] // 128, axis=0)
    else:  # fallback: no structure assumed
        lo = np.zeros(NT, np.int64)
        hi = np.full(NT, CT - 1, np.int64)
    covered = set()
    for t in range(NT):
        covered.update(range(int(lo[t]), int(hi[t]) + 1))
    if covered != set(range(CT)):
        lo = np.zeros(NT, np.int64)
        hi = np.full(NT, CT - 1, np.int64)
    ranges = tuple((int(lo[t]), int(hi[t])) for t in range(NT))

    # attention row-broadcast selectors: rb2[s] rows 0:64 <- rcp row 32*(2s),
    # rows 64:128 <- rcp row 32*(2s+1)
    sel = np.zeros((2, 128, 128), np.float32)
    sel[0, 0, 0:64] = 1.0
    sel[0, 32, 64:128] = 1.0
    sel[1, 64, 0:64] = 1.0
    sel[1, 96, 64:128] = 1.0

    # shared (batch-independent) arrays
    shared = {
        "sel2": sel,
        "iota_row": np.broadcast_to(
            np.arange(C, dtype=np.float16)[None, :], (128, C)).copy(),
        "iota_col": (np.arange(CT, dtype=np.float32)[None, :] * 128
                     + np.arange(128, dtype=np.float32)[:, None]).astype(np.float32),
        "wqkvT": np.ascontiguousarray(qkv_w.transpose(0, 2, 1)).astype(bf),
        "woT": np.ascontiguousarray(out_w.transpose(0, 2, 1)).astype(bf),
        "w1T": np.ascontiguousarray(ff1_w.transpose(0, 2, 1)).astype(bf),
        "w2T": np.ascontiguousarray(ff2_w.transpose(0, 2, 1)).astype(bf),
    }
    (has_qkv_b, has_out_b, has_ff1_b, has_ff2_b,
     has_ln1, has_ln2, has_fln) = flags
    if has_qkv_b:
        shared["bqkv_c"] = np.ascontiguousarray(
            qkv_b[:, :1536].reshape(L, 12, 128).transpose(0, 2, 1))
        shared["vb_row"] = np.ascontiguousarray(qkv_b[:, 2 * D:3 * D][:, None, :])
    if has_ff1_b:
        shared["b1_c"] = np.ascontiguousarray(
            ff1_b.reshape(L, FT, 128).transpose(0, 2, 1))
    if has_out_b:
        shared["outb_row"] = np.ascontiguousarray(out_b[:, None, :])
    if has_ff2_b:
        shared["ff2b_row"] = np.ascontiguousarray(ff2_b[:, None, :])
    if has_ln1:
        shared["ln1w_row"] = np.ascontiguousarray(ln1_w[:, None, :])
        shared["ln1b_row"] = np.ascontiguousarray(ln1_b[:, None, :])
    if has_ln2:
        shared["ln2w_row"] = np.ascontiguousarray(ln2_w[:, None, :])
        shared["ln2b_row"] = np.ascontiguousarray(ln2_b[:, None, :])
    if has_fln:
        shared["flnw_row"] = np.ascontiguousarray(fln_w[None, :])
        shared["flnb_row"] = np.ascontiguousarray(fln_b[None, :])

    in_maps = []
    for b in range(B):
        m = dict(shared)
        m["tokens_bf"] = np.ascontiguousarray(tokens[b]).astype(bf)
        m["seg_col"] = np.ascontiguousarray(
            seg[b].reshape(NT, 128).T.astype(np.float32))
        m["seg_row"] = np.ascontiguousarray(seg[b].astype(np.float16)[None, :])
        cnt = np.bincount(seg[b], minlength=C).astype(np.float32)
        cnt[cnt == 0] = 1.0
        m["rcp_cnt"] = np.ascontiguousarray(
            (1.0 / cnt).reshape(CT, 128).T.astype(np.float32))
        in_maps.append(m)
    return flags, ranges, in_maps


def kernel(**inputs) -> np.ndarray:
    from concourse.bass_utils import run_bass_kernel_spmd

    flags, ranges, in_maps = _host_prep(inputs)
    key = (flags, ranges)
    if key not in _CACHE:
        nc = _build(flags, ranges)
        if not nc.is_finalized():
            nc.finalize()
        _CACHE[key] = nc
    nc = _CACHE[key]
    res = run_bass_kernel_spmd(nc, in_maps, list(range(B)))
    return np.stack([res.results[i]["out"] for i in range(B)], axis=0)


# revision 32
# speedup vs baseline: 1.0663x; 1.0663x over previous
"""Trainium2 Bass kernel for nn_ChunkProcessor (segment-mean -> 2-layer
transformer encoder over chunks -> gather-expand -> final LN).

Sharding: data-parallel over batch B=8 across the 8 NeuronCores; each core
processes one batch item end to end (no cross-core communication).

v2 vs baseline:
  - tokens + all weights host-cast to bf16 (halves phase-1 HBM traffic).
  - segment counts' reciprocals host-computed (kills 85 tiny PE matmuls).
  - one ACT table set for the whole kernel (natural_log_exp_and_others):
    LN rsqrt = exp(-0.5*ln(var+eps)), softmax denom recip = exp(-ln(x)),
    so no Exp<->Sqrt table thrash between attention and LayerNorm.
  - attention: fused [128,1024] exps, denominator rows batched per 4-head
    group with one ACT ln/exp reciprocal, row-broadcast via GpSimd
    partition_broadcast, numerators multiplied straight out of PSUM.
  - expand one-hots prebuilt during phase 2 (DVE slack), weights single-
    buffered in SBUF with layer-1 refill DMA'd during layer-0 compute.
"""

import numpy as np
import ml_dtypes

B, S, D = 8, 8192, 512
C, H, L, DFF = 512, 8, 2, 2048
HD = D // H          # 64
NT = S // 128        # 64 token tiles
CT = C // 128        # 4 chunk tiles
DT = D // 128        # 4 feature tiles
FT = DFF // 128      # 16
EPS = 1e-5

_CACHE = {}
_DEBUG = False


def _gm_slots(ranges):
    """(group, chunk-tile) pairs the expand phase touches, in emit order."""
    slots = []
    for g in range(NT // 4):
        g_lo = min(ranges[g * 4 + i][0] for i in range(4))
        g_hi = max(ranges[g * 4 + i][1] for i in range(4))
        for m in range(g_lo, g_hi + 1):
            slots.append((g, m))
    return slots


def _build(flags, ranges):
    import concourse.bass as bass
    import concourse.tile as tile
    from concourse import bacc, mybir
    from concourse.masks import make_identity

    (has_qkv_b, has_out_b, has_ff1_b, has_ff2_b,
     has_ln1, has_ln2, has_fln) = flags

    # first/last contributing token tile per chunk tile (PSUM start/stop)
    first_t = [min(t for t in range(NT) if ranges[t][0] <= m <= ranges[t][1])
               for m in range(CT)]
    last_t = [max(t for t in range(NT) if ranges[t][0] <= m <= ranges[t][1])
              for m in range(CT)]

    gm = _gm_slots(ranges)
    slot = {p: i for i, p in enumerate(gm)}
    n_gm = len(gm)
    prebuild_oh = n_gm <= 48

    f32 = mybir.dt.float32
    bf16 = mybir.dt.bfloat16
    f16 = mybir.dt.float16
    AL = mybir.AluOpType
    AF = mybir.ActivationFunctionType

    nc = bacc.Bacc("TRN2", target_bir_lowering=False)

    tokens = nc.declare_dram_parameter("tokens_bf", [S, D], bf16, isOutput=False)
    seg_col = nc.declare_dram_parameter("seg_col", [128, NT], f32, isOutput=False)
    seg_row = nc.declare_dram_parameter("seg_row", [1, S], f16, isOutput=False)
    iota_row = nc.declare_dram_parameter("iota_row", [128, C], f16, isOutput=False)
    iota_col = nc.declare_dram_parameter("iota_col", [128, CT], f32, isOutput=False)
    rcp_cnt = nc.declare_dram_parameter("rcp_cnt", [128, CT], f32, isOutput=False)
    sel2 = nc.declare_dram_parameter("sel2", [2, 128, 128], f32, isOutput=False)
    wqkvT = nc.declare_dram_parameter("wqkvT", [L, D, 3 * D], bf16, isOutput=False)
    woT = nc.declare_dram_parameter("woT", [L, D, D], bf16, isOutput=False)
    w1T = nc.declare_dram_parameter("w1T", [L, D, DFF], bf16, isOutput=False)
    w2T = nc.declare_dram_parameter("w2T", [L, DFF, D], bf16, isOutput=False)
    if has_qkv_b:
        bqkv_c = nc.declare_dram_parameter("bqkv_c", [L, 128, 12], f32, isOutput=False)
        vb_row = nc.declare_dram_parameter("vb_row", [L, 1, D], f32, isOutput=False)
    if has_ff1_b:
        b1_c = nc.declare_dram_parameter("b1_c", [L, 128, FT], f32, isOutput=False)
    if has_out_b:
        outb_row = nc.declare_dram_parameter("outb_row", [L, 1, D], f32, isOutput=False)
    if has_ff2_b:
        ff2b_row = nc.declare_dram_parameter("ff2b_row", [L, 1, D], f32, isOutput=False)
    if has_ln1:
        ln1w_row = nc.declare_dram_parameter("ln1w_row", [L, 1, D], f32, isOutput=False)
        ln1b_row = nc.declare_dram_parameter("ln1b_row", [L, 1, D], f32, isOutput=False)
    if has_ln2:
        ln2w_row = nc.declare_dram_parameter("ln2w_row", [L, 1, D], f32, isOutput=False)
        ln2b_row = nc.declare_dram_parameter("ln2b_row", [L, 1, D], f32, isOutput=False)
    if has_fln:
        flnw_row = nc.declare_dram_parameter("flnw_row", [1, D], f32, isOutput=False)
        flnb_row = nc.declare_dram_parameter("flnb_row", [1, D], f32, isOutput=False)
    out_d = nc.declare_dram_parameter("out", [S, D], f32, isOutput=True)
    if _DEBUG:
        dbg_x0 = nc.declare_dram_parameter("dbg_x0", [128, CT, D], f32,
                                           isOutput=True)
        dbg_xm2 = nc.declare_dram_parameter("dbg_xm2", [128, CT, D], f32,
                                            isOutput=True)
        dbg_oT = nc.declare_dram_parameter("dbg_oT", [128, DT, C], bf16,
                                           isOutput=True)
        dbg_x1 = nc.declare_dram_parameter("dbg_x1", [128, CT, D], f32,
                                           isOutput=True)
        dbg_x2T = nc.declare_dram_parameter("dbg_x2T", [128, DT, C], bf16,
                                            isOutput=True)
        dbg_hT = nc.declare_dram_parameter("dbg_hT", [128, FT, C], bf16,
                                           isOutput=True)

    def bcast_load(pool, dram_row, tag):
        """DMA a [1, D] DRAM row into a [128, D] SBUF tile (partition bcast)."""
        t = pool.tile([128, D], f32, tag=tag, name=f"row_{tag}")
        src = bass.AP(tensor=dram_row.tensor, offset=dram_row.offset,
                      ap=[[0, 128]] + [list(p) for p in dram_row.ap[1:]])
        nc.gpsimd.dma_start(out=t, in_=src)
        return t

    with tile.TileContext(nc) as tc:
        with (
            tc.tile_pool(name="consts", bufs=1) as consts,
            tc.tile_pool(name="acts", bufs=1) as acts,
            tc.tile_pool(name="xm", bufs=2) as xmp,
            tc.tile_pool(name="xt", bufs=2) as xtp,
            tc.tile_pool(name="lnp", bufs=4) as lnp,
            tc.tile_pool(name="rows", bufs=1) as rows,
            tc.tile_pool(name="wts", bufs=1) as wts,
        ):
            # ---------------- constants ----------------
            seg_col_sb = consts.tile([128, NT], f32)
            nc.sync.dma_start(out=seg_col_sb, in_=seg_col[:, :])
            iota_row_sb = consts.tile([128, C], f16)
            nc.sync.dma_start(out=iota_row_sb, in_=iota_row[:, :])
            iota_col_sb = consts.tile([128, CT], f32)
            nc.sync.dma_start(out=iota_col_sb, in_=iota_col[:, :])
            rcp_sb = consts.tile([128, CT], f32)
            nc.sync.dma_start(out=rcp_sb, in_=rcp_cnt[:, :])
            sel_sb = consts.tile([128, 2, 128], f32)
            nc.sync.dma_start(
                out=sel_sb, in_=sel2.rearrange("s p e -> p s e"))
            ident32 = consts.tile([128, 128], f32)
            make_identity(nc, ident32)
            eps_t = consts.tile([128, 1], f32)
            nc.vector.memset(eps_t, EPS)

            # persistent activations
            y_bf = acts.tile([128, CT, D], bf16, tag="y_bf")
            segbc_all = acts.tile([128, NT // 4, 512], f16, tag="segbc")
            if prebuild_oh:
                oh_all = acts.tile([128, n_gm, 512], bf16, tag="oh_all")

            # ---- weights: single-buffered, layer 0 now (scalar HWDGE) ----
            wqkv_sb = wts.tile([128, DT, 3 * D], bf16, tag="wqkv")
            wo_sb = wts.tile([128, DT, D], bf16, tag="wo")
            w1_sb = wts.tile([128, DT, DFF], bf16, tag="w1")
            w2_sb = wts.tile([128, FT, D], bf16, tag="w2")

            def load_weights(l, eng):
                eng.dma_start(
                    out=wqkv_sb, in_=wqkvT[l].rearrange("(dt p) e -> p dt e", p=128))
                eng.dma_start(
                    out=wo_sb, in_=woT[l].rearrange("(dt p) e -> p dt e", p=128))
                eng.dma_start(
                    out=w1_sb, in_=w1T[l].rearrange("(dt p) e -> p dt e", p=128))
                eng.dma_start(
                    out=w2_sb, in_=w2T[l].rearrange("(ft p) e -> p ft e", p=128))

            load_weights(0, nc.scalar)

            # seg broadcast rows for the expand one-hots (SWDGE, idle queue)
            for g in range(NT // 4):
                src = bass.AP(tensor=seg_row[:, :].tensor, offset=g * 512,
                              ap=[[0, 128], [1, 512]])
                nc.gpsimd.dma_start(out=segbc_all[:, g, :], in_=src)

            bqkv_sb, b1_sb = [], []
            vb_sb, outb_sb, ff2b_sb = [], [], []
            ln1w_sb, ln1b_sb, ln2w_sb, ln2b_sb = [], [], [], []
            for l in range(L):
                if has_qkv_b:
                    bq = consts.tile([128, 12], f32, tag=f"bqkv{l}", name=f"bqkv{l}")
                    nc.sync.dma_start(out=bq, in_=bqkv_c[l])
                    bqkv_sb.append(bq)
                    vb_sb.append(bcast_load(rows, vb_row[l], f"vb{l}"))
                if has_ff1_b:
                    b1 = consts.tile([128, FT], f32, tag=f"b1{l}", name=f"b1{l}")
                    nc.sync.dma_start(out=b1, in_=b1_c[l])
                    b1_sb.append(b1)
                if has_out_b:
                    outb_sb.append(bcast_load(rows, outb_row[l], f"outb{l}"))
                if has_ff2_b:
                    ff2b_sb.append(bcast_load(rows, ff2b_row[l], f"ff2b{l}"))
                if has_ln1:
                    ln1w_sb.append(bcast_load(rows, ln1w_row[l], f"ln1w{l}"))
                    ln1b_sb.append(bcast_load(rows, ln1b_row[l], f"ln1b{l}"))
                if has_ln2:
                    ln2w_sb.append(bcast_load(rows, ln2w_row[l], f"ln2w{l}"))
                    ln2b_sb.append(bcast_load(rows, ln2b_row[l], f"ln2b{l}"))
            flnw_sb = bcast_load(rows, flnw_row, "flnw") if has_fln else None
            flnb_sb = bcast_load(rows, flnb_row, "flnb") if has_fln else None

            # ---- batched-LN site: 4 chunk tiles, rsqrt via exp(-.5*ln) ----
            def ln_site(srcs, resids, wrow, brow, outs):
                """outs[m] = LN(srcs[m] (+ resids[m])) [* w + b], token-major."""
                ts, mv4 = [], lnp.tile([128, 2, CT], f32, tag="mv4", name="mv4")
                for m in range(CT):
                    if resids is None:
                        t_ = srcs[m]
                    else:
                        t_ = lnp.tile([128, D], f32, tag="ln_t", name=f"ln_t{m}")
                        nc.vector.tensor_tensor(out=t_, in0=srcs[m],
                                                in1=resids[m], op=AL.add)
                    ts.append(t_)
                    st = lnp.tile([128, 6], f32, tag="ln_st", name="ln_st")
                    nc.vector.bn_stats(out=st, in_=t_)
                    nc.vector.bn_aggr(out=mv4[:, :, m], in_=st)
                # rs = exp(-0.5 * ln(var + eps)) on the contiguous var slice
                ln4 = lnp.tile([128, CT], f32, tag="ln4", name="ln4")
                nc.scalar.activation(out=ln4, in_=mv4[:, 1, :], func=AF.Ln,
                                     bias=eps_t[:, 0:1], scale=1.0)
                rs4 = lnp.tile([128, CT], f32, tag="rs4", name="rs4")
                nc.scalar.activation(out=rs4, in_=ln4, func=AF.Exp,
                                     bias=0.0, scale=-0.5)
                for m in range(CT):
                    if wrow is None:
                        nc.vector.tensor_scalar(
                            out=outs[m], in0=ts[m],
                            scalar1=mv4[:, 0, m:m + 1],
                            scalar2=rs4[:, m:m + 1],
                            op0=AL.subtract, op1=AL.mult)
                    else:
                        xn = lnp.tile([128, D], f32, tag="ln_xn", name="ln_xn")
                        nc.vector.tensor_scalar(
                            out=xn, in0=ts[m],
                            scalar1=mv4[:, 0, m:m + 1],
                            scalar2=rs4[:, m:m + 1],
                            op0=AL.subtract, op1=AL.mult)
                        nc.vector.tensor_tensor(out=xn, in0=xn, in1=wrow, op=AL.mult)
                        nc.vector.tensor_tensor(out=outs[m], in0=xn, in1=brow,
                                                op=AL.add)

            # ------------ phase 1: segment sums (counts from host) ------------
            x0 = xmp.tile([128, CT, D], f32, tag="xm", name="x0")
            with (
                tc.tile_pool(name="pseg", bufs=1, space="PSUM") as pseg,
                tc.tile_pool(name="segs", bufs=5) as segs,
            ):
                ps_sums = [pseg.tile([128, D], f32, tag=f"sums{m}", name=f"sums{m}")
                           for m in range(CT)]
                for t in range(NT):
                    lo, hi = ranges[t]
                    tok = segs.tile([128, D], bf16, tag="tok", name="tok")
                    nc.sync.dma_start(out=tok, in_=tokens[t * 128:(t + 1) * 128, :])
                    oh = segs.tile([128, C], bf16, tag="oh", name="oh")
                    sl = slice(lo * 128, (hi + 1) * 128)
                    nc.vector.tensor_scalar(
                        out=oh[:, sl], in0=iota_row_sb[:, sl],
                        scalar1=seg_col_sb[:, t:t + 1],
                        scalar2=None, op0=AL.is_equal)
                    for m in range(lo, hi + 1):
                        nc.tensor.matmul(
                            ps_sums[m], lhsT=oh[:, m * 128:(m + 1) * 128], rhs=tok,
                            start=(t == first_t[m]), stop=(t == last_t[m]))
                # x = sums * (1/counts)  (host-computed reciprocals)
                for m in range(CT):
                    nc.vector.tensor_scalar(
                        out=x0[:, m, :], in0=ps_sums[m],
                        scalar1=rcp_sb[:, m:m + 1],
                        scalar2=None, op0=AL.mult)

            # one-hot prebuild batches, interleaved at DVE-slack points
            oh_batches = [[] for _ in range(4)]
            if prebuild_oh:
                for i, p in enumerate(gm):
                    oh_batches[i * 4 // n_gm].append(p)

            def emit_oh(batch):
                for (g, m) in batch:
                    nc.vector.tensor_scalar(
                        out=oh_all[:, slot[(g, m)], :], in0=segbc_all[:, g, :],
                        scalar1=iota_col_sb[:, m:m + 1],
                        scalar2=None, op0=AL.is_equal)

            # ---------------- phase 2: transformer ----------------
            x_in = x0
            if _DEBUG:
                nc.sync.dma_start(out=dbg_x0[:, :, :], in_=x0[:, :, :])
            for l in range(L):
                # --- stage A: transposes + qkv projections ---
                with tc.tile_pool(name=f"psqA{l}", bufs=2, space="PSUM") as psq:
                    def transpose_to(src_f32, dst_bf16):
                        for i in range(CT):
                            for j in range(DT):
                                pst = psq.tile([128, 128], f32, tag="ps_t",
                                               name="ps_t")
                                nc.tensor.transpose(
                                    pst, src_f32[:, i, j * 128:(j + 1) * 128],
                                    ident32)
                                nc.vector.tensor_copy(
                                    dst_bf16[:, j, i * 128:(i + 1) * 128], pst)

                    xT = xtp.tile([128, DT, C], bf16, tag="xT", name="xT")
                    transpose_to(x_in, xT)

                    # v token-major [c, e] with per-head ones column (DVE copies)
                    v_ext = acts.tile([128, CT, H, HD + 1], bf16, tag="v_ext",
                                      name="v_ext")
                    nc.vector.memset(v_ext[:, :, :, HD:HD + 1], 1.0)
                    for ct in range(CT):
                        ps = psq.tile([128, C], f32, tag="ps_a", name="ps_a")
                        for dt_ in range(DT):
                            nc.tensor.matmul(
                                ps, lhsT=xT[:, dt_, ct * 128:(ct + 1) * 128],
                                rhs=wqkv_sb[:, dt_, 2 * D:3 * D],
                                start=(dt_ == 0), stop=(dt_ == DT - 1))
                        if has_qkv_b:
                            tv = lnp.tile([128, D], f32, tag="ln_t", name="tv")
                            nc.vector.tensor_tensor(out=tv, in0=ps, in1=vb_sb[l],
                                                    op=AL.add)
                            nc.vector.tensor_copy(v_ext[:, ct, :, 0:HD], tv)
                        else:
                            nc.vector.tensor_copy(v_ext[:, ct, :, 0:HD], ps)

                    # q, k feature-major [e, c] (ACT copies)
                    qT = acts.tile([128, DT, C], bf16, tag="qT", name="qT")
                    kT = acts.tile([128, DT, C], bf16, tag="kT", name="kT")
                    for et in range(8):
                        ps = psq.tile([128, C], f32, tag="ps_a", name="ps_a")
                        for dt_ in range(DT):
                            nc.tensor.matmul(
                                ps, lhsT=wqkv_sb[:, dt_, et * 128:(et + 1) * 128],
                                rhs=xT[:, dt_, :],
                                start=(dt_ == 0), stop=(dt_ == DT - 1))
                        dst = qT[:, et, :] if et < 4 else kT[:, et - 4, :]
                        if has_qkv_b:
                            nc.scalar.activation(
                                out=dst, in_=ps, func=AF.Identity,
                                bias=bqkv_sb[l][:, et:et + 1], scale=1.0)
                        else:
                            nc.scalar.copy(out=dst, in_=ps)

                # --- stage B: attention ---
                oT = acts.tile([128, DT, C], bf16, tag="oT", name="oT")
                with (
                    tc.tile_pool(name=f"pssc{l}", bufs=4, space="PSUM") as scp,
                    tc.tile_pool(name=f"psso{l}", bufs=4, space="PSUM") as psop,
                ):
                    for grp in range(2):
                        psos = []
                        # denominator rows restriped to partitions 0/32/64/96
                        # (DVE partition offsets must be 32-aligned); memset
                        # keeps the junk rows finite through ln/exp
                        den4 = acts.tile([128, C], f32, tag="den4",
                                         name="den4", bufs=2)
                        rcp4 = acts.tile([128, C], f32, tag="rcp4",
                                         name="rcp4", bufs=2)
                        nc.vector.memset(den4, 1.0)
                        for hh in range(4):
                            h = grp * 4 + hh
                            th, off = h // 2, (h % 2) * 64
                            expT = acts.tile([128, CT * C], bf16, tag="expT",
                                             name="expT", bufs=2)
                            for kt in range(CT):
                                ps = scp.tile([128, 512], f32, tag="ps_sc",
                                              name="ps_sc")
                                nc.tensor.matmul(
                                    ps,
                                    lhsT=kT[off:off + 64, th,
                                            kt * 128:(kt + 1) * 128],
                                    rhs=qT[off:off + 64, th, :],
                                    start=True, stop=True)
                                nc.scalar.activation(
                                    out=expT[:, kt * 512:(kt + 1) * 512],
                                    in_=ps, func=AF.Exp, scale=1.0 / 8.0)
                            pso = psop.tile([128, C], f32, tag="ps_o", name="ps_o")
                            for kt in range(CT):
                                nc.tensor.matmul(
                                    pso[0:HD + 1, :], lhsT=v_ext[:, kt, h, :],
                                    rhs=expT[:, kt * 512:(kt + 1) * 512],
                                    start=(kt == 0), stop=(kt == CT - 1))
                            nc.vector.tensor_copy(den4[32 * hh:32 * hh + 1, :],
                                                  pso[HD:HD + 1, :])
                            psos.append(pso)
                        # reciprocal of the 4 den rows: exp(-ln(x))
                        nc.scalar.activation(out=rcp4, in_=den4,
                                             func=AF.Ln, scale=1.0)
                        nc.scalar.activation(out=rcp4, in_=rcp4,
                                             func=AF.Exp, scale=-1.0)
                        # row-broadcast via selector matmuls: rb2[s] rows
                        # 0:64 <- rcp row 32*(2s), 64:128 <- rcp row 32*(2s+1)
                        rb2 = []
                        for s_ in range(2):
                            rb_ps = scp.tile([128, 512], f32, tag="ps_sc",
                                             name="rb_ps")
                            nc.tensor.matmul(rb_ps, lhsT=sel_sb[:, s_, :],
                                             rhs=rcp4, start=True, stop=True)
                            rb = acts.tile([128, C], f32, tag="rb2",
                                           name="rb2", bufs=4)
                            nc.vector.tensor_copy(rb, rb_ps)
                            rb2.append(rb)
                        for hh in range(4):
                            h = grp * 4 + hh
                            th, off = h // 2, (h % 2) * 64
                            nc.vector.tensor_tensor(
                                out=oT[off:off + 64, th, :], in0=psos[hh][0:HD, :],
                                in1=rb2[hh // 2][(hh % 2) * 64:(hh % 2) * 64 + 64, :],
                                op=AL.mult)

                emit_oh(oh_batches[0] if l == 0 else oh_batches[2])

                # --- stage C: out-projection + LN1 + FFN + LN2 ---
                with tc.tile_pool(name=f"psqC{l}", bufs=2, space="PSUM") as psq:
                    xm2 = xmp.tile([128, CT, D], f32, tag="xm", name="xm2")
                    o_ps = []
                    for ct in range(CT):
                        ps = psq.tile([128, C], f32, tag="ps_a", name="ps_a",
                                      bufs=4)
                        for et in range(DT):
                            nc.tensor.matmul(
                                ps, lhsT=oT[:, et, ct * 128:(ct + 1) * 128],
                                rhs=wo_sb[:, et, :],
                                start=(et == 0), stop=(et == DT - 1))
                        if has_out_b:
                            nc.vector.tensor_tensor(out=ps, in0=ps,
                                                    in1=outb_sb[l], op=AL.add)
                        o_ps.append(ps)
                    ln_site(o_ps, [x_in[:, ct, :] for ct in range(CT)],
                            ln1w_sb[l] if has_ln1 else None,
                            ln1b_sb[l] if has_ln1 else None,
                            [xm2[:, ct, :] for ct in range(CT)])

                    def transpose_to2(src_f32, dst_bf16):
                        for i in range(CT):
                            for j in range(DT):
                                pst = psq.tile([128, 128], f32, tag="ps_t",
                                               name="ps_t")
                                nc.tensor.transpose(
                                    pst, src_f32[:, i, j * 128:(j + 1) * 128],
                                    ident32)
                                nc.vector.tensor_copy(
                                    dst_bf16[:, j, i * 128:(i + 1) * 128], pst)

                    x2T = xtp.tile([128, DT, C], bf16, tag="xT", name="x2T")
                    transpose_to2(xm2, x2T)

                    hT = acts.tile([128, FT, C], bf16, tag="hT", name="hT")
                    for ft in range(FT):
                        ps = psq.tile([128, C], f32, tag="ps_a", name="ps_a",
                                      bufs=4)
                        for dt_ in range(DT):
                            nc.tensor.matmul(
                                ps, lhsT=w1_sb[:, dt_, ft * 128:(ft + 1) * 128],
                                rhs=x2T[:, dt_, :],
                                start=(dt_ == 0), stop=(dt_ == DT - 1))
                        if ft % 2 == 0:
                            nc.scalar.activation(
                                out=hT[:, ft, :], in_=ps, func=AF.Relu,
                                bias=(b1_sb[l][:, ft:ft + 1] if has_ff1_b else 0.0),
                                scale=1.0)
                        else:
                            if has_ff1_b:
                                nc.scalar.activation(
                                    out=hT[:, ft, :], in_=ps, func=AF.Relu,
                                    bias=b1_sb[l][:, ft:ft + 1], scale=1.0)
                            else:
                                nc.vector.tensor_scalar(
                                    out=hT[:, ft, :], in0=ps, scalar1=0.0,
                                    scalar2=None, op0=AL.max)
                    x_next = xmp.tile([128, CT, D], f32, tag="xm", name="x_next")
                    f_ps = []
                    for ct in range(CT):
                        ps = psq.tile([128, C], f32, tag="ps_a", name="ps_a",
                                      bufs=4)
                        for ft in range(FT):
                            nc.tensor.matmul(
                                ps, lhsT=hT[:, ft, ct * 128:(ct + 1) * 128],
                                rhs=w2_sb[:, ft, :],
                                start=(ft == 0), stop=(ft == FT - 1))
                        if has_ff2_b:
                            nc.vector.tensor_tensor(out=ps, in0=ps,
                                                    in1=ff2b_sb[l], op=AL.add)
                        f_ps.append(ps)
                    ln_site(f_ps, [xm2[:, ct, :] for ct in range(CT)],
                            ln2w_sb[l] if has_ln2 else None,
                            ln2b_sb[l] if has_ln2 else None,
                            [x_next[:, ct, :] for ct in range(CT)])
                    x_in = x_next
                if _DEBUG and l == 0:
                    nc.sync.dma_start(out=dbg_oT[:, :, :], in_=oT[:, :, :])
                    nc.sync.dma_start(out=dbg_xm2[:, :, :], in_=xm2[:, :, :])
                    nc.sync.dma_start(out=dbg_x1[:, :, :], in_=x_next[:, :, :])
                    nc.sync.dma_start(out=dbg_x2T[:, :, :], in_=x2T[:, :, :])
                    nc.sync.dma_start(out=dbg_hT[:, :, :], in_=hT[:, :, :])

                if l == 0:
                    # refill for layer 1 on the SWDGE queue — emitted after
                    # ALL layer-0 weight reads (out-proj + ffn just above)
                    load_weights(1, nc.gpsimd)
                emit_oh(oh_batches[1] if l == 0 else oh_batches[3])

            # ---------------- phase 3: final LN -> y_bf ----------------
            ln_site([x_in[:, ct, :] for ct in range(CT)], None,
                    flnw_sb, flnb_sb,
                    [y_bf[:, ct, :] for ct in range(CT)])

            # ---------------- expand + write out ----------------
            with (
                tc.tile_pool(name="ohp", bufs=4) as ohp,
                tc.tile_pool(name="outp", bufs=4) as outp,
                tc.tile_pool(name="psE", bufs=4, space="PSUM") as psE,
            ):
                def expand_tile(t, oh_lookup):
                    g, t2 = t // 4, t % 4
                    lo, hi = ranges[t]
                    pse = psE.tile([128, D], f32, tag="ps_e", name="ps_e")
                    for m in range(lo, hi + 1):
                        nc.tensor.matmul(
                            pse, lhsT=oh_lookup(g, m, t2), rhs=y_bf[:, m, :],
                            start=(m == lo), stop=(m == hi))
                    ot = outp.tile([128, D], f32, tag="ot", name="ot")
                    if t % 2 == 0:
                        nc.scalar.copy(out=ot, in_=pse)
                    else:
                        nc.vector.tensor_copy(ot, pse)
                    nc.sync.dma_start(out=out_d[t * 128:(t + 1) * 128, :],
                                      in_=ot)

                if prebuild_oh:
                    # order token tiles by the last chunk tile they need, so
                    # expand for hi==m streams as soon as y_bf[:, m, :] lands
                    for t in sorted(range(NT), key=lambda t: (ranges[t][1], t)):
                        expand_tile(t, lambda g, m, t2: oh_all[
                            :, slot[(g, m)], t2 * 128:(t2 + 1) * 128])
                else:
                    for g in range(NT // 4):
                        g_lo = min(ranges[g * 4 + i][0] for i in range(4))
                        g_hi = max(ranges[g * 4 + i][1] for i in range(4))
                        ohT = ohp.tile([128, CT, 512], bf16, tag="ohT",
                                       name="ohT")
                        for m in range(g_lo, g_hi + 1):
                            nc.vector.tensor_scalar(
                                out=ohT[:, m, :], in0=segbc_all[:, g, :],
                                scalar1=iota_col_sb[:, m:m + 1], scalar2=None,
                                op0=AL.is_equal)
                        for t2 in range(4):
                            expand_tile(g * 4 + t2, lambda g_, m, t2_: ohT[
                                :, m, t2_ * 128:(t2_ + 1) * 128])

    return nc


def _host_prep(inputs):
    """Shard + preprocess full inputs into 8 per-core input maps."""
    bf = ml_dtypes.bfloat16
    tokens = np.asarray(inputs["tokens"], dtype=np.float32)
    seg = np.asarray(inputs["segment_ids"], dtype=np.int32)
    qkv_w = np.asarray(inputs["qkv_w"], dtype=np.float32)
    qkv_b = np.asarray(inputs["qkv_b"], dtype=np.float32)
    out_w = np.asarray(inputs["out_w"], dtype=np.float32)
    out_b = np.asarray(inputs["out_b"], dtype=np.float32)
    ln1_w = np.asarray(inputs["ln1_w"], dtype=np.float32)
    ln1_b = np.asarray(inputs["ln1_b"], dtype=np.float32)
    ln2_w = np.asarray(inputs["ln2_w"], dtype=np.float32)
    ln2_b = np.asarray(inputs["ln2_b"], dtype=np.float32)
    ff1_w = np.asarray(inputs["ff1_w"], dtype=np.float32)
    ff1_b = np.asarray(inputs["ff1_b"], dtype=np.float32)
    ff2_w = np.asarray(inputs["ff2_w"], dtype=np.float32)
    ff2_b = np.asarray(inputs["ff2_b"], dtype=np.float32)
    fln_w = np.asarray(inputs["fln_w"], dtype=np.float32)
    fln_b = np.asarray(inputs["fln_b"], dtype=np.float32)

    flags = (
        bool(np.any(qkv_b)),
        bool(np.any(out_b)),
        bool(np.any(ff1_b)),
        bool(np.any(ff2_b)),
        bool(np.any(ln1_w != 1.0) or np.any(ln1_b)),
        bool(np.any(ln2_w != 1.0) or np.any(ln2_b)),
        bool(np.any(fln_w != 1.0) or np.any(fln_b)),
    )

    # span-bound ranges: per token tile, union over batch of the contiguous
    # chunk-tile range its (sorted) segment ids cover.
    srt = np.all(np.diff(seg, axis=1) >= 0)
    if srt:
        lo = np.min(seg[:, ::128] // 128, axis=0)
        hi = np.max(seg[:, 127::128] // 128, axis=0)
    else:  # fallback: no structure assumed
        lo = np.zeros(NT, np.int64)
        hi = np.full(NT, CT - 1, np.int64)
    covered = set()
    for t in range(NT):
        covered.update(range(int(lo[t]), int(hi[t]) + 1))
    if covered != set(range(CT)):
        lo = np.zeros(NT, np.int64)
        hi = np.full(NT, CT - 1, np.int64)
    ranges = tuple((int(lo[t]), int(hi[t])) for t in range(NT))

    # attention row-broadcast selectors: rb2[s] rows 0:64 <- rcp row 32*(2s),
    # rows 64:128 <- rcp row 32*(2s+1)
    sel = np.zeros((2, 128, 128), np.float32)
    sel[0, 0, 0:64] = 1.0
    sel[0, 32, 64:128] = 1.0
    sel[1, 64, 0:64] = 1.0
    sel[1, 96, 64:128] = 1.0

    # shared (batch-independent) arrays
    shared = {
        "sel2": sel,
        "iota_row": np.broadcast_to(
            np.arange(C, dtype=np.float16)[None, :], (128, C)).copy(),
        "iota_col": (np.arange(CT, dtype=np.float32)[None, :] * 128
                     + np.arange(128, dtype=np.float32)[:, None]).astype(np.float32),
        "wqkvT": np.ascontiguousarray(qkv_w.transpose(0, 2, 1)).astype(bf),
        "woT": np.ascontiguousarray(out_w.transpose(0, 2, 1)).astype(bf),
        "w1T": np.ascontiguousarray(ff1_w.transpose(0, 2, 1)).astype(bf),
        "w2T": np.ascontiguousarray(ff2_w.transpose(0, 2, 1)).astype(bf),
    }
    (has_qkv_b, has_out_b, has_ff1_b, has_ff2_b,
     has_ln1, has_ln2, has_fln) = flags
    if has_qkv_b:
        shared["bqkv_c"] = np.ascontiguousarray(
            qkv_b[:, :1536].reshape(L, 12, 128).transpose(0, 2, 1))
        shared["vb_row"] = np.ascontiguousarray(qkv_b[:, 2 * D:3 * D][:, None, :])
    if has_ff1_b:
        shared["b1_c"] = np.ascontiguousarray(
            ff1_b.reshape(L, FT, 128).transpose(0, 2, 1))
    if has_out_b:
        shared["outb_row"] = np.ascontiguousarray(out_b[:, None, :])
    if has_ff2_b:
        shared["ff2b_row"] = np.ascontiguousarray(ff2_b[:, None, :])
    if has_ln1:
        shared["ln1w_row"] = np.ascontiguousarray(ln1_w[:, None, :])
        shared["ln1b_row"] = np.ascontiguousarray(ln1_b[:, None, :])
    if has_ln2:
        shared["ln2w_row"] = np.ascontiguousarray(ln2_w[:, None, :])
        shared["ln2b_row"] = np.ascontiguousarray(ln2_b[:, None, :])
    if has_fln:
        shared["flnw_row"] = np.ascontiguousarray(fln_w[None, :])
        shared["flnb_row"] = np.ascontiguousarray(fln_b[None, :])

    in_maps = []
    for b in range(B):
        m = dict(shared)
        m["tokens_bf"] = np.ascontiguousarray(tokens[b]).astype(bf)
        m["seg_col"] = np.ascontiguousarray(
            seg[b].reshape(NT, 128).T.astype(np.float32))
        m["seg_row"] = np.ascontiguousarray(seg[b].astype(np.float16)[None, :])
        cnt = np.bincount(seg[b], minlength=C).astype(np.float32)
        cnt[cnt == 0] = 1.0
        m["rcp_cnt"] = np.ascontiguousarray(
            (1.0 / cnt).reshape(CT, 128).T.astype(np.float32))
        in_maps.append(m)
    return flags, ranges, in_maps


def kernel(**inputs) -> np.ndarray:
    from concourse.bass_utils import run_bass_kernel_spmd

    flags, ranges, in_maps = _host_prep(inputs)
    key = (flags, ranges)
    if key not in _CACHE:
        nc = _build(flags, ranges)
        if not nc.is_finalized():
            nc.finalize()
        _CACHE[key] = nc
    nc = _CACHE[key]
    res = run_bass_kernel_spmd(nc, in_maps, list(range(B)))
    return np.stack([res.results[i]["out"] for i in range(B)], axis=0)


# revision 37
# speedup vs baseline: 1.1198x; 1.0502x over previous
"""Trainium2 Bass kernel for nn_ChunkProcessor (segment-mean -> 2-layer
transformer encoder over chunks -> gather-expand -> final LN).

Sharding: data-parallel over batch B=8 across the 8 NeuronCores; each core
processes one batch item end to end (no cross-core communication).

v2 vs baseline:
  - tokens + all weights host-cast to bf16 (halves phase-1 HBM traffic).
  - segment counts' reciprocals host-computed (kills 85 tiny PE matmuls).
  - one ACT table set for the whole kernel (natural_log_exp_and_others):
    LN rsqrt = exp(-0.5*ln(var+eps)), softmax denom recip = exp(-ln(x)),
    so no Exp<->Sqrt table thrash between attention and LayerNorm.
  - attention: fused [128,1024] exps, denominator rows batched per 4-head
    group with one ACT ln/exp reciprocal, row-broadcast via GpSimd
    partition_broadcast, numerators multiplied straight out of PSUM.
  - expand one-hots prebuilt during phase 2 (DVE slack), weights single-
    buffered in SBUF with layer-1 refill DMA'd during layer-0 compute.
"""

import numpy as np
import ml_dtypes

B, S, D = 8, 8192, 512
C, H, L, DFF = 512, 8, 2, 2048
HD = D // H          # 64
NT = S // 128        # 64 token tiles
CT = C // 128        # 4 chunk tiles
DT = D // 128        # 4 feature tiles
FT = DFF // 128      # 16
EPS = 1e-5

_CACHE = {}
_DEBUG = False
# fp8 scope: 'both' | 'ff1' | 'none'
_FP8_MODE = 'ff1'


def _gm_slots(ranges):
    """(group, chunk-tile) pairs the expand phase touches, in emit order."""
    slots = []
    for g in range(NT // 4):
        g_lo = min(ranges[g * 4 + i][0] for i in range(4))
        g_hi = max(ranges[g * 4 + i][1] for i in range(4))
        for m in range(g_lo, g_hi + 1):
            slots.append((g, m))
    return slots


def _build(flags, ranges):
    import concourse.bass as bass
    import concourse.tile as tile
    from concourse import bacc, mybir
    from concourse.masks import make_identity

    (has_qkv_b, has_out_b, has_ff1_b, has_ff2_b,
     has_ln1, has_ln2, has_fln) = flags

    # first/last contributing token tile per chunk tile (PSUM start/stop)
    first_t = [min(t for t in range(NT) if ranges[t][0] <= m <= ranges[t][1])
               for m in range(CT)]
    last_t = [max(t for t in range(NT) if ranges[t][0] <= m <= ranges[t][1])
              for m in range(CT)]

    gm = _gm_slots(ranges)
    slot = {p: i for i, p in enumerate(gm)}
    n_gm = len(gm)
    prebuild_oh = n_gm <= 48

    f32 = mybir.dt.float32
    bf16 = mybir.dt.bfloat16
    f16 = mybir.dt.float16
    f8 = mybir.dt.float8e4
    AL = mybir.AluOpType
    AF = mybir.ActivationFunctionType
    DR = mybir.MatmulPerfMode.DoubleRow
    # fp8 DoubleRow FFN (weights pre-scaled x64 on host); f16 fallback
    # when ff biases are present (scale folding not wired for that path)
    fp8_ok = not (has_ff1_b or has_ff2_b)
    fp8_1 = fp8_ok and _FP8_MODE in ('both', 'ff1')
    fp8_2 = fp8_ok and _FP8_MODE == 'both'
    ff1dt = f8 if fp8_1 else f16
    ff2dt = f8 if fp8_2 else f16

    nc = bacc.Bacc("TRN2", target_bir_lowering=False)

    tokens = nc.declare_dram_parameter("tokens_bf", [S, D], f16, isOutput=False)
    seg_col = nc.declare_dram_parameter("seg_col", [128, NT], f32, isOutput=False)
    seg_row = nc.declare_dram_parameter("seg_row", [1, S], f16, isOutput=False)
    iota_row = nc.declare_dram_parameter("iota_row", [128, C], f16, isOutput=False)
    iota_col = nc.declare_dram_parameter("iota_col", [128, CT], f32, isOutput=False)
    rcp_cnt = nc.declare_dram_parameter("rcp_cnt", [128, CT], f32, isOutput=False)
    sel2 = nc.declare_dram_parameter("sel2", [2, 128, 128], f32, isOutput=False)
    wqkvT = nc.declare_dram_parameter("wqkvT", [L, D, 3 * D], f16, isOutput=False)
    woT = nc.declare_dram_parameter("woT", [L, D, D], f16, isOutput=False)
    w1T = nc.declare_dram_parameter("w1T", [L, D, DFF], ff1dt, isOutput=False)
    w2T = nc.declare_dram_parameter("w2T", [L, DFF, D], ff2dt, isOutput=False)
    if has_qkv_b:
        bqkv_c = nc.declare_dram_parameter("bqkv_c", [L, 128, 12], f32, isOutput=False)
        vb_row = nc.declare_dram_parameter("vb_row", [L, 1, D], f32, isOutput=False)
    if has_ff1_b:
        b1_c = nc.declare_dram_parameter("b1_c", [L, 128, FT], f32, isOutput=False)
    if has_out_b:
        outb_row = nc.declare_dram_parameter("outb_row", [L, 1, D], f32, isOutput=False)
    if has_ff2_b:
        ff2b_row = nc.declare_dram_parameter("ff2b_row", [L, 1, D], f32, isOutput=False)
    if has_ln1:
        ln1w_row = nc.declare_dram_parameter("ln1w_row", [L, 1, D], f32, isOutput=False)
        ln1b_row = nc.declare_dram_parameter("ln1b_row", [L, 1, D], f32, isOutput=False)
    if has_ln2:
        ln2w_row = nc.declare_dram_parameter("ln2w_row", [L, 1, D], f32, isOutput=False)
        ln2b_row = nc.declare_dram_parameter("ln2b_row", [L, 1, D], f32, isOutput=False)
    if has_fln:
        flnw_row = nc.declare_dram_parameter("flnw_row", [1, D], f32, isOutput=False)
        flnb_row = nc.declare_dram_parameter("flnb_row", [1, D], f32, isOutput=False)
    out_d = nc.declare_dram_parameter("out", [S, D], f32, isOutput=True)
    if _DEBUG:
        dbg_x0 = nc.declare_dram_parameter("dbg_x0", [128, CT, D], f32,
                                           isOutput=True)
        dbg_xm2 = nc.declare_dram_parameter("dbg_xm2", [128, CT, D], f32,
                                            isOutput=True)
        dbg_oT = nc.declare_dram_parameter("dbg_oT", [128, DT, C], f16,
                                           isOutput=True)
        dbg_x1 = nc.declare_dram_parameter("dbg_x1", [128, CT, D], f32,
                                           isOutput=True)
        dbg_x2T = nc.declare_dram_parameter("dbg_x2T", [128, DT, C], ff1dt,
                                            isOutput=True)
        dbg_hT = nc.declare_dram_parameter("dbg_hT", [128, FT, C], ff2dt,
                                           isOutput=True)

    def bcast_load(pool, dram_row, tag):
        """DMA a [1, D] DRAM row into a [128, D] SBUF tile (partition bcast)."""
        t = pool.tile([128, D], f32, tag=tag, name=f"row_{tag}")
        src = bass.AP(tensor=dram_row.tensor, offset=dram_row.offset,
                      ap=[[0, 128]] + [list(p) for p in dram_row.ap[1:]])
        nc.gpsimd.dma_start(out=t, in_=src)
        return t

    with tile.TileContext(nc) as tc:
        with (
            tc.tile_pool(name="consts", bufs=1) as consts,
            tc.tile_pool(name="acts", bufs=1) as acts,
            tc.tile_pool(name="xm", bufs=2) as xmp,
            tc.tile_pool(name="xt", bufs=2) as xtp,
            tc.tile_pool(name="lnp", bufs=4) as lnp,
            tc.tile_pool(name="rows", bufs=1) as rows,
            tc.tile_pool(name="wts", bufs=1) as wts,
        ):
            # ---------------- constants ----------------
            seg_col_sb = consts.tile([128, NT], f32)
            nc.sync.dma_start(out=seg_col_sb, in_=seg_col[:, :])
            iota_row_sb = consts.tile([128, C], f16)
            nc.sync.dma_start(out=iota_row_sb, in_=iota_row[:, :])
            iota_col_sb = consts.tile([128, CT], f32)
            nc.sync.dma_start(out=iota_col_sb, in_=iota_col[:, :])
            rcp_sb = consts.tile([128, CT], f32)
            nc.sync.dma_start(out=rcp_sb, in_=rcp_cnt[:, :])
            sel_sb = consts.tile([128, 2, 128], f32)
            nc.sync.dma_start(
                out=sel_sb, in_=sel2.rearrange("s p e -> p s e"))
            ident32 = consts.tile([128, 128], f32)
            make_identity(nc, ident32)
            eps_t = consts.tile([128, 1], f32)
            nc.vector.memset(eps_t, EPS)

            # persistent activations
            y_bf = acts.tile([128, CT, D], f16, tag="y_bf")
            segbc_all = acts.tile([128, NT // 4, 512], f16, tag="segbc")
            if prebuild_oh:
                oh_all = acts.tile([128, n_gm, 512], f16, tag="oh_all")

            # ---- weights: single-buffered, layer 0 now (scalar HWDGE) ----
            wqkv_sb = wts.tile([128, DT, 3 * D], f16, tag="wqkv")
            wo_sb = wts.tile([128, DT, D], f16, tag="wo")
            w1_sb = wts.tile([128, DT, DFF], ff1dt, tag="w1")
            w2_sb = wts.tile([128, FT, D], ff2dt, tag="w2")

            def load_weights(l, eng):
                eng.dma_start(
                    out=wqkv_sb, in_=wqkvT[l].rearrange("(dt p) e -> p dt e", p=128))
                eng.dma_start(
                    out=wo_sb, in_=woT[l].rearrange("(dt p) e -> p dt e", p=128))
                eng.dma_start(
                    out=w1_sb, in_=w1T[l].rearrange("(dt p) e -> p dt e", p=128))
                eng.dma_start(
                    out=w2_sb, in_=w2T[l].rearrange("(ft p) e -> p ft e", p=128))

            load_weights(0, nc.scalar)

            # seg broadcast rows for the expand one-hots (SWDGE, idle queue)
            for g in range(NT // 4):
                src = bass.AP(tensor=seg_row[:, :].tensor, offset=g * 512,
                              ap=[[0, 128], [1, 512]])
                nc.gpsimd.dma_start(out=segbc_all[:, g, :], in_=src)

            bqkv_sb, b1_sb = [], []
            vb_sb, outb_sb, ff2b_sb = [], [], []
            ln1w_sb, ln1b_sb, ln2w_sb, ln2b_sb = [], [], [], []
            for l in range(L):
                if has_qkv_b:
                    bq = consts.tile([128, 12], f32, tag=f"bqkv{l}", name=f"bqkv{l}")
                    nc.sync.dma_start(out=bq, in_=bqkv_c[l])
                    bqkv_sb.append(bq)
                    vb_sb.append(bcast_load(rows, vb_row[l], f"vb{l}"))
                if has_ff1_b:
                    b1 = consts.tile([128, FT], f32, tag=f"b1{l}", name=f"b1{l}")
                    nc.sync.dma_start(out=b1, in_=b1_c[l])
                    b1_sb.append(b1)
                if has_out_b:
                    outb_sb.append(bcast_load(rows, outb_row[l], f"outb{l}"))
                if has_ff2_b:
                    ff2b_sb.append(bcast_load(rows, ff2b_row[l], f"ff2b{l}"))
                if has_ln1:
                    ln1w_sb.append(bcast_load(rows, ln1w_row[l], f"ln1w{l}"))
                    ln1b_sb.append(bcast_load(rows, ln1b_row[l], f"ln1b{l}"))
                if has_ln2:
                    ln2w_sb.append(bcast_load(rows, ln2w_row[l], f"ln2w{l}"))
                    ln2b_sb.append(bcast_load(rows, ln2b_row[l], f"ln2b{l}"))
            flnw_sb = bcast_load(rows, flnw_row, "flnw") if has_fln else None
            flnb_sb = bcast_load(rows, flnb_row, "flnb") if has_fln else None

            # ---- batched-LN site: 4 chunk tiles, rsqrt via exp(-.5*ln) ----
            def ln_site(srcs, resids, wrow, brow, outs, src_scale=None):
                """outs[m] = LN(srcs[m]*s (+ resids[m])) [* w + b]."""
                ts, mv4 = [], lnp.tile([128, 2, CT], f32, tag="mv4", name="mv4")
                for m in range(CT):
                    if resids is None:
                        t_ = srcs[m]
                    else:
                        t_ = lnp.tile([128, D], f32, tag="ln_t", name=f"ln_t{m}")
                        if src_scale is None:
                            nc.vector.tensor_tensor(out=t_, in0=srcs[m],
                                                    in1=resids[m], op=AL.add)
                        else:
                            nc.vector.scalar_tensor_tensor(
                                out=t_, in0=srcs[m], scalar=src_scale,
                                in1=resids[m], op0=AL.mult, op1=AL.add)
                    ts.append(t_)
                    st = lnp.tile([128, 6], f32, tag="ln_st", name="ln_st")
                    nc.vector.bn_stats(out=st, in_=t_)
                    nc.vector.bn_aggr(out=mv4[:, :, m], in_=st)
                # rs = exp(-0.5 * ln(var + eps)) on the contiguous var slice
                ln4 = lnp.tile([128, CT], f32, tag="ln4", name="ln4")
                nc.scalar.activation(out=ln4, in_=mv4[:, 1, :], func=AF.Ln,
                                     bias=eps_t[:, 0:1], scale=1.0)
                rs4 = lnp.tile([128, CT], f32, tag="rs4", name="rs4")
                nc.scalar.activation(out=rs4, in_=ln4, func=AF.Exp,
                                     bias=0.0, scale=-0.5)
                for m in range(CT):
                    if wrow is None:
                        nc.vector.tensor_scalar(
                            out=outs[m], in0=ts[m],
                            scalar1=mv4[:, 0, m:m + 1],
                            scalar2=rs4[:, m:m + 1],
                            op0=AL.subtract, op1=AL.mult)
                    else:
                        xn = lnp.tile([128, D], f32, tag="ln_xn", name="ln_xn")
                        nc.vector.tensor_scalar(
                            out=xn, in0=ts[m],
                            scalar1=mv4[:, 0, m:m + 1],
                            scalar2=rs4[:, m:m + 1],
                            op0=AL.subtract, op1=AL.mult)
                        nc.vector.tensor_tensor(out=xn, in0=xn, in1=wrow, op=AL.mult)
                        nc.vector.tensor_tensor(out=outs[m], in0=xn, in1=brow,
                                                op=AL.add)

            # ------------ phase 1: segment sums (counts from host) ------------
            x0 = xmp.tile([128, CT, D], f32, tag="xm", name="x0")
            with (
                tc.tile_pool(name="pseg", bufs=1, space="PSUM") as pseg,
                tc.tile_pool(name="segs", bufs=5) as segs,
            ):
                ps_sums = [pseg.tile([128, D], f32, tag=f"sums{m}", name=f"sums{m}")
                           for m in range(CT)]
                for t in range(NT):
                    lo, hi = ranges[t]
                    tok = segs.tile([128, D], f16, tag="tok", name="tok")
                    nc.sync.dma_start(out=tok, in_=tokens[t * 128:(t + 1) * 128, :])
                    oh = segs.tile([128, C], f16, tag="oh", name="oh")
                    sl = slice(lo * 128, (hi + 1) * 128)
                    nc.vector.tensor_scalar(
                        out=oh[:, sl], in0=iota_row_sb[:, sl],
                        scalar1=seg_col_sb[:, t:t + 1],
                        scalar2=None, op0=AL.is_equal)
                    for m in range(lo, hi + 1):
                        nc.tensor.matmul(
                            ps_sums[m], lhsT=oh[:, m * 128:(m + 1) * 128], rhs=tok,
                            start=(t == first_t[m]), stop=(t == last_t[m]))
                # x = sums * (1/counts)  (host-computed reciprocals)
                for m in range(CT):
                    nc.vector.tensor_scalar(
                        out=x0[:, m, :], in0=ps_sums[m],
                        scalar1=rcp_sb[:, m:m + 1],
                        scalar2=None, op0=AL.mult)

            # one-hot prebuild batches, interleaved at DVE-slack points
            oh_batches = [[] for _ in range(4)]
            if prebuild_oh:
                for i, p in enumerate(gm):
                    oh_batches[i * 4 // n_gm].append(p)

            def emit_oh(batch):
                for (g, m) in batch:
                    nc.vector.tensor_scalar(
                        out=oh_all[:, slot[(g, m)], :], in0=segbc_all[:, g, :],
                        scalar1=iota_col_sb[:, m:m + 1],
                        scalar2=None, op0=AL.is_equal)

            # ---------------- phase 2: transformer ----------------
            x_in = x0
            if _DEBUG:
                nc.sync.dma_start(out=dbg_x0[:, :, :], in_=x0[:, :, :])
            for l in range(L):
                # --- stage A: transposes + qkv projections ---
                with tc.tile_pool(name=f"psqA{l}", bufs=2, space="PSUM") as psq:
                    def transpose_to(src_f32, dst_bf16):
                        for i in range(CT):
                            for j in range(DT):
                                pst = psq.tile([128, 128], f32, tag="ps_t",
                                               name="ps_t")
                                nc.tensor.transpose(
                                    pst, src_f32[:, i, j * 128:(j + 1) * 128],
                                    ident32)
                                nc.vector.tensor_copy(
                                    dst_bf16[:, j, i * 128:(i + 1) * 128], pst)

                    xT = xtp.tile([128, DT, C], f16, tag="xT", name="xT")
                    transpose_to(x_in, xT)

                    # v token-major [c, e] with per-head ones column (DVE copies)
                    v_ext = acts.tile([128, CT, H, HD + 1], f16, tag="v_ext",
                                      name="v_ext")
                    nc.vector.memset(v_ext[:, :, :, HD:HD + 1], 1.0)
                    for ct in range(CT):
                        ps = psq.tile([128, C], f32, tag="ps_a", name="ps_a")
                        for dt_ in range(DT):
                            nc.tensor.matmul(
                                ps, lhsT=xT[:, dt_, ct * 128:(ct + 1) * 128],
                                rhs=wqkv_sb[:, dt_, 2 * D:3 * D],
                                start=(dt_ == 0), stop=(dt_ == DT - 1))
                        if has_qkv_b:
                            tv = lnp.tile([128, D], f32, tag="ln_t", name="tv")
                            nc.vector.tensor_tensor(out=tv, in0=ps, in1=vb_sb[l],
                                                    op=AL.add)
                            nc.vector.tensor_copy(v_ext[:, ct, :, 0:HD], tv)
                        else:
                            nc.vector.tensor_copy(v_ext[:, ct, :, 0:HD], ps)

                    # q, k feature-major [e, c] (ACT copies)
                    qT = acts.tile([128, DT, C], f16, tag="qT", name="qT")
                    kT = acts.tile([128, DT, C], f16, tag="kT", name="kT")
                    for et in range(8):
                        ps = psq.tile([128, C], f32, tag="ps_a", name="ps_a")
                        for dt_ in range(DT):
                            nc.tensor.matmul(
                                ps, lhsT=wqkv_sb[:, dt_, et * 128:(et + 1) * 128],
                                rhs=xT[:, dt_, :],
                                start=(dt_ == 0), stop=(dt_ == DT - 1))
                        dst = qT[:, et, :] if et < 4 else kT[:, et - 4, :]
                        if has_qkv_b:
                            nc.scalar.activation(
                                out=dst, in_=ps, func=AF.Identity,
                                bias=bqkv_sb[l][:, et:et + 1], scale=1.0)
                        else:
                            nc.scalar.copy(out=dst, in_=ps)

                # --- stage B: attention ---
                oT = acts.tile([128, DT, C], f16, tag="oT", name="oT")
                with (
                    tc.tile_pool(name=f"pssc{l}", bufs=4, space="PSUM") as scp,
                    tc.tile_pool(name=f"psso{l}", bufs=4, space="PSUM") as psop,
                ):
                    for grp in range(2):
                        psos = []
                        # denominator rows restriped to partitions 0/32/64/96
                        # (DVE partition offsets must be 32-aligned); memset
                        # keeps the junk rows finite through ln/exp
                        den4 = acts.tile([128, C], f32, tag="den4",
                                         name="den4", bufs=2)
                        rcp4 = acts.tile([128, C], f32, tag="rcp4",
                                         name="rcp4", bufs=2)
                        nc.vector.memset(den4, 1.0)
                        for hh in range(4):
                            h = grp * 4 + hh
                            th, off = h // 2, (h % 2) * 64
                            expT = acts.tile([128, CT * C], f16, tag="expT",
                                             name="expT", bufs=2)
                            for kt in range(CT):
                                ps = scp.tile([128, 512], f32, tag="ps_sc",
                                              name="ps_sc")
                                nc.tensor.matmul(
                                    ps,
                                    lhsT=kT[off:off + 64, th,
                                            kt * 128:(kt + 1) * 128],
                                    rhs=qT[off:off + 64, th, :],
                                    start=True, stop=True)
                                nc.scalar.activation(
                                    out=expT[:, kt * 512:(kt + 1) * 512],
                                    in_=ps, func=AF.Exp, scale=1.0 / 8.0)
                            pso = psop.tile([128, C], f32, tag="ps_o", name="ps_o")
                            for kt in range(CT):
                                nc.tensor.matmul(
                                    pso[0:HD + 1, :], lhsT=v_ext[:, kt, h, :],
                                    rhs=expT[:, kt * 512:(kt + 1) * 512],
                                    start=(kt == 0), stop=(kt == CT - 1))
                            nc.vector.tensor_copy(den4[32 * hh:32 * hh + 1, :],
                                                  pso[HD:HD + 1, :])
                            psos.append(pso)
                        # reciprocal of the 4 den rows: exp(-ln(x))
                        nc.scalar.activation(out=rcp4, in_=den4,
                                             func=AF.Ln, scale=1.0)
                        nc.scalar.activation(out=rcp4, in_=rcp4,
                                             func=AF.Exp, scale=-1.0)
                        # row-broadcast via selector matmuls: rb2[s] rows
                        # 0:64 <- rcp row 32*(2s), 64:128 <- rcp row 32*(2s+1)
                        rb2 = []
                        for s_ in range(2):
                            rb_ps = scp.tile([128, 512], f32, tag="ps_sc",
                                             name="rb_ps")
                            nc.tensor.matmul(rb_ps, lhsT=sel_sb[:, s_, :],
                                             rhs=rcp4, start=True, stop=True)
                            rb = acts.tile([128, C], f32, tag="rb2",
                                           name="rb2", bufs=4)
                            nc.vector.tensor_copy(rb, rb_ps)
                            rb2.append(rb)
                        for hh in range(4):
                            h = grp * 4 + hh
                            th, off = h // 2, (h % 2) * 64
                            nc.vector.tensor_tensor(
                                out=oT[off:off + 64, th, :], in0=psos[hh][0:HD, :],
                                in1=rb2[hh // 2][(hh % 2) * 64:(hh % 2) * 64 + 64, :],
                                op=AL.mult)

                emit_oh(oh_batches[0] if l == 0 else oh_batches[2])

                # --- stage C: out-projection + LN1 + FFN + LN2 ---
                with tc.tile_pool(name=f"psqC{l}", bufs=2, space="PSUM") as psq:
                    xm2 = xmp.tile([128, CT, D], f32, tag="xm", name="xm2")
                    o_ps = []
                    for ct in range(CT):
                        ps = psq.tile([128, C], f32, tag="ps_a", name="ps_a",
                                      bufs=4)
                        for et in range(DT):
                            nc.tensor.matmul(
                                ps, lhsT=oT[:, et, ct * 128:(ct + 1) * 128],
                                rhs=wo_sb[:, et, :],
                                start=(et == 0), stop=(et == DT - 1))
                        if has_out_b:
                            nc.vector.tensor_tensor(out=ps, in0=ps,
                                                    in1=outb_sb[l], op=AL.add)
                        o_ps.append(ps)
                    ln_site(o_ps, [x_in[:, ct, :] for ct in range(CT)],
                            ln1w_sb[l] if has_ln1 else None,
                            ln1b_sb[l] if has_ln1 else None,
                            [xm2[:, ct, :] for ct in range(CT)])

                    def transpose_to2(src_f32, dst_bf16):
                        for i in range(CT):
                            for j in range(DT):
                                pst = psq.tile([128, 128], f32, tag="ps_t",
                                               name="ps_t")
                                nc.tensor.transpose(
                                    pst, src_f32[:, i, j * 128:(j + 1) * 128],
                                    ident32)
                                nc.vector.tensor_copy(
                                    dst_bf16[:, j, i * 128:(i + 1) * 128], pst)

                    x2T = xtp.tile([128, DT, C], ff1dt, tag="xT", name="x2T")
                    transpose_to2(xm2, x2T)

                    hT = acts.tile([128, FT, C], ff2dt, tag="hT", name="hT")
                    rl_scale = (1.0 / 64.0) if fp8_1 else 1.0
                    for ft in range(FT):
                        ps = psq.tile([128, C], f32, tag="ps_a", name="ps_a",
                                      bufs=4)
                        if fp8_1:
                            for p2 in range(0, DT, 2):
                                nc.tensor.matmul(
                                    ps,
                                    lhsT=w1_sb[:, p2:p2 + 2,
                                               ft * 128:(ft + 1) * 128],
                                    rhs=x2T[:, p2:p2 + 2, :],
                                    start=(p2 == 0), stop=(p2 == DT - 2),
                                    perf_mode=DR)
                        else:
                            for dt_ in range(DT):
                                nc.tensor.matmul(
                                    ps,
                                    lhsT=w1_sb[:, dt_, ft * 128:(ft + 1) * 128],
                                    rhs=x2T[:, dt_, :],
                                    start=(dt_ == 0), stop=(dt_ == DT - 1))
                        if ft % 2 == 0:
                            nc.scalar.activation(
                                out=hT[:, ft, :], in_=ps, func=AF.Relu,
                                bias=(b1_sb[l][:, ft:ft + 1] if has_ff1_b else 0.0),
                                scale=rl_scale)
                        else:
                            if has_ff1_b:
                                nc.scalar.activation(
                                    out=hT[:, ft, :], in_=ps, func=AF.Relu,
                                    bias=b1_sb[l][:, ft:ft + 1], scale=rl_scale)
                            else:
                                nc.vector.tensor_scalar(
                                    out=hT[:, ft, :], in0=ps, scalar1=0.0,
                                    scalar2=rl_scale, op0=AL.max, op1=AL.mult)
                    x_next = xmp.tile([128, CT, D], f32, tag="xm", name="x_next")
                    f_ps = []
                    for ct in range(CT):
                        ps = psq.tile([128, C], f32, tag="ps_a", name="ps_a",
                                      bufs=4)
                        if fp8_2:
                            for p2 in range(0, FT, 2):
                                nc.tensor.matmul(
                                    ps,
                                    lhsT=hT[:, p2:p2 + 2,
                                            ct * 128:(ct + 1) * 128],
                                    rhs=w2_sb[:, p2:p2 + 2, :],
                                    start=(p2 == 0), stop=(p2 == FT - 2),
                                    perf_mode=DR)
                        else:
                            for ft in range(FT):
                                nc.tensor.matmul(
                                    ps, lhsT=hT[:, ft, ct * 128:(ct + 1) * 128],
                                    rhs=w2_sb[:, ft, :],
                                    start=(ft == 0), stop=(ft == FT - 1))
                        if has_ff2_b:
                            nc.vector.tensor_tensor(out=ps, in0=ps,
                                                    in1=ff2b_sb[l], op=AL.add)
                        f_ps.append(ps)
                    ln_site(f_ps, [xm2[:, ct, :] for ct in range(CT)],
                            ln2w_sb[l] if has_ln2 else None,
                            ln2b_sb[l] if has_ln2 else None,
                            [x_next[:, ct, :] for ct in range(CT)],
                            src_scale=(1.0 / 64.0) if fp8_2 else None)
                    x_in = x_next
                if _DEBUG and l == 0:
                    nc.sync.dma_start(out=dbg_oT[:, :, :], in_=oT[:, :, :])
                    nc.sync.dma_start(out=dbg_xm2[:, :, :], in_=xm2[:, :, :])
                    nc.sync.dma_start(out=dbg_x1[:, :, :], in_=x_next[:, :, :])
                    nc.sync.dma_start(out=dbg_x2T[:, :, :], in_=x2T[:, :, :])
                    nc.sync.dma_start(out=dbg_hT[:, :, :], in_=hT[:, :, :])

                if l == 0:
                    # refill for layer 1 on the SWDGE queue — emitted after
                    # ALL layer-0 weight reads (out-proj + ffn just above)
                    load_weights(1, nc.gpsimd)
                emit_oh(oh_batches[1] if l == 0 else oh_batches[3])

            # ---------------- phase 3: final LN -> y_bf ----------------
            ln_site([x_in[:, ct, :] for ct in range(CT)], None,
                    flnw_sb, flnb_sb,
                    [y_bf[:, ct, :] for ct in range(CT)])

            # ---------------- expand + write out ----------------
            with (
                tc.tile_pool(name="ohp", bufs=4) as ohp,
                tc.tile_pool(name="outp", bufs=4) as outp,
                tc.tile_pool(name="psE", bufs=4, space="PSUM") as psE,
            ):
                def expand_tile(t, oh_lookup):
                    g, t2 = t // 4, t % 4
                    lo, hi = ranges[t]
                    pse = psE.tile([128, D], f32, tag="ps_e", name="ps_e")
                    for m in range(lo, hi + 1):
                        nc.tensor.matmul(
                            pse, lhsT=oh_lookup(g, m, t2), rhs=y_bf[:, m, :],
                            start=(m == lo), stop=(m == hi))
                    ot = outp.tile([128, D], f32, tag="ot", name="ot")
                    if t % 2 == 0:
                        nc.scalar.copy(out=ot, in_=pse)
                    else:
                        nc.vector.tensor_copy(ot, pse)
                    nc.sync.dma_start(out=out_d[t * 128:(t + 1) * 128, :],
                                      in_=ot)

                if prebuild_oh:
                    # order token tiles by the last chunk tile they need, so
                    # expand for hi==m streams as soon as y_bf[:, m, :] lands
                    for t in sorted(range(NT), key=lambda t: (ranges[t][1], t)):
                        expand_tile(t, lambda g, m, t2: oh_all[
                            :, slot[(g, m)], t2 * 128:(t2 + 1) * 128])
                else:
                    for g in range(NT // 4):
                        g_lo = min(ranges[g * 4 + i][0] for i in range(4))
                        g_hi = max(ranges[g * 4 + i][1] for i in range(4))
                        ohT = ohp.tile([128, CT, 512], f16, tag="ohT",
                                       name="ohT")
                        for m in range(g_lo, g_hi + 1):
                            nc.vector.tensor_scalar(
                                out=ohT[:, m, :], in0=segbc_all[:, g, :],
                                scalar1=iota_col_sb[:, m:m + 1], scalar2=None,
                                op0=AL.is_equal)
                        for t2 in range(4):
                            expand_tile(g * 4 + t2, lambda g_, m, t2_: ohT[
                                :, m, t2_ * 128:(t2_ + 1) * 128])

    return nc


def _host_prep(inputs):
    """Shard + preprocess full inputs into 8 per-core input maps."""
    bf = ml_dtypes.bfloat16
    tokens = np.asarray(inputs["tokens"], dtype=np.float32)
    seg = np.asarray(inputs["segment_ids"], dtype=np.int32)
    qkv_w = np.asarray(inputs["qkv_w"], dtype=np.float32)
    qkv_b = np.asarray(inputs["qkv_b"], dtype=np.float32)
    out_w = np.asarray(inputs["out_w"], dtype=np.float32)
    out_b = np.asarray(inputs["out_b"], dtype=np.float32)
    ln1_w = np.asarray(inputs["ln1_w"], dtype=np.float32)
    ln1_b = np.asarray(inputs["ln1_b"], dtype=np.float32)
    ln2_w = np.asarray(inputs["ln2_w"], dtype=np.float32)
    ln2_b = np.asarray(inputs["ln2_b"], dtype=np.float32)
    ff1_w = np.asarray(inputs["ff1_w"], dtype=np.float32)
    ff1_b = np.asarray(inputs["ff1_b"], dtype=np.float32)
    ff2_w = np.asarray(inputs["ff2_w"], dtype=np.float32)
    ff2_b = np.asarray(inputs["ff2_b"], dtype=np.float32)
    fln_w = np.asarray(inputs["fln_w"], dtype=np.float32)
    fln_b = np.asarray(inputs["fln_b"], dtype=np.float32)

    flags = (
        bool(np.any(qkv_b)),
        bool(np.any(out_b)),
        bool(np.any(ff1_b)),
        bool(np.any(ff2_b)),
        bool(np.any(ln1_w != 1.0) or np.any(ln1_b)),
        bool(np.any(ln2_w != 1.0) or np.any(ln2_b)),
        bool(np.any(fln_w != 1.0) or np.any(fln_b)),
    )

    # span-bound ranges: per token tile, union over batch of the contiguous
    # chunk-tile range its (sorted) segment ids cover.
    srt = np.all(np.diff(seg, axis=1) >= 0)
    if srt:
        lo = np.min(seg[:, ::128] // 128, axis=0)
        hi = np.max(seg[:, 127::128] // 128, axis=0)
    else:  # fallback: no structure assumed
        lo = np.zeros(NT, np.int64)
        hi = np.full(NT, CT - 1, np.int64)
    covered = set()
    for t in range(NT):
        covered.update(range(int(lo[t]), int(hi[t]) + 1))
    if covered != set(range(CT)):
        lo = np.zeros(NT, np.int64)
        hi = np.full(NT, CT - 1, np.int64)
    ranges = tuple((int(lo[t]), int(hi[t])) for t in range(NT))

    # attention row-broadcast selectors: rb2[s] rows 0:64 <- rcp row 32*(2s),
    # rows 64:128 <- rcp row 32*(2s+1)
    sel = np.zeros((2, 128, 128), np.float32)
    sel[0, 0, 0:64] = 1.0
    sel[0, 32, 64:128] = 1.0
    sel[1, 64, 0:64] = 1.0
    sel[1, 96, 64:128] = 1.0

    # shared (batch-independent) arrays
    shared = {
        "sel2": sel,
        "iota_row": np.broadcast_to(
            np.arange(C, dtype=np.float16)[None, :], (128, C)).copy(),
        "iota_col": (np.arange(CT, dtype=np.float32)[None, :] * 128
                     + np.arange(128, dtype=np.float32)[:, None]).astype(np.float32),
        "wqkvT": np.ascontiguousarray(qkv_w.transpose(0, 2, 1)).astype(np.float16),
        "woT": np.ascontiguousarray(out_w.transpose(0, 2, 1)).astype(np.float16),
    }
    fp8_ok = not (bool(np.any(ff1_b)) or bool(np.any(ff2_b)))
    f8 = ml_dtypes.float8_e4m3
    if fp8_ok and _FP8_MODE in ('both', 'ff1'):
        # fp8e4m3 weights, pre-scaled x64 so |w| sits in the normal range
        shared["w1T"] = np.ascontiguousarray(
            ff1_w.transpose(0, 2, 1) * 64.0).astype(f8)
    else:
        shared["w1T"] = np.ascontiguousarray(
            ff1_w.transpose(0, 2, 1)).astype(np.float16)
    if fp8_ok and _FP8_MODE == 'both':
        shared["w2T"] = np.ascontiguousarray(
            ff2_w.transpose(0, 2, 1) * 64.0).astype(f8)
    else:
        shared["w2T"] = np.ascontiguousarray(
            ff2_w.transpose(0, 2, 1)).astype(np.float16)
    (has_qkv_b, has_out_b, has_ff1_b, has_ff2_b,
     has_ln1, has_ln2, has_fln) = flags
    if has_qkv_b:
        shared["bqkv_c"] = np.ascontiguousarray(
            qkv_b[:, :1536].reshape(L, 12, 128).transpose(0, 2, 1))
        shared["vb_row"] = np.ascontiguousarray(qkv_b[:, 2 * D:3 * D][:, None, :])
    if has_ff1_b:
        shared["b1_c"] = np.ascontiguousarray(
            ff1_b.reshape(L, FT, 128).transpose(0, 2, 1))
    if has_out_b:
        shared["outb_row"] = np.ascontiguousarray(out_b[:, None, :])
    if has_ff2_b:
        shared["ff2b_row"] = np.ascontiguousarray(ff2_b[:, None, :])
    if has_ln1:
        shared["ln1w_row"] = np.ascontiguousarray(ln1_w[:, None, :])
        shared["ln1b_row"] = np.ascontiguousarray(ln1_b[:, None, :])
    if has_ln2:
        shared["ln2w_row"] = np.ascontiguousarray(ln2_w[:, None, :])
        shared["ln2b_row"] = np.ascontiguousarray(ln2_b[:, None, :])
    if has_fln:
        shared["flnw_row"] = np.ascontiguousarray(fln_w[None, :])
        shared["flnb_row"] = np.ascontiguousarray(fln_b[None, :])

    in_maps = []
    for b in range(B):
        m = dict(shared)
        m["tokens_bf"] = np.ascontiguousarray(tokens[b]).astype(np.float16)
        m["seg_col"] = np.ascontiguousarray(
            seg[b].reshape(NT, 128).T.astype(np.float32))
        m["seg_row"] = np.ascontiguousarray(seg[b].astype(np.float16)[None, :])
        cnt = np.bincount(seg[b], minlength=C).astype(np.float32)
        cnt[cnt == 0] = 1.0
        m["rcp_cnt"] = np.ascontiguousarray(
            (1.0 / cnt).reshape(CT, 128).T.astype(np.float32))
        in_maps.append(m)
    return flags, ranges, in_maps


def kernel(**inputs) -> np.ndarray:
    from concourse.bass_utils import run_bass_kernel_spmd

    flags, ranges, in_maps = _host_prep(inputs)
    key = (flags, ranges, _FP8_MODE)
    if key not in _CACHE:
        nc = _build(flags, ranges)
        if not nc.is_finalized():
            nc.finalize()
        _CACHE[key] = nc
    nc = _CACHE[key]
    res = run_bass_kernel_spmd(nc, in_maps, list(range(B)))
    return np.stack([res.results[i]["out"] for i in range(B)], axis=0)
